# revision 8
# baseline (speedup 1.0000x reference)
"""Trainium2 Bass kernel for BiDACPI (GAT + CNN + bidirectional attention).

Data-parallel over batch b=16 across 8 NeuronCores (2 graphs per core).
Self-contained: hardcodes all shapes; host-side preprocessing only reshapes /
transposes weights and converts index tensors.
"""
import numpy as np

import concourse.bass as bass
import concourse.mybir as mybir
import concourse.tile as tile
from concourse import bacc

F32 = mybir.dt.float32
I32 = mybir.dt.int32
AT = mybir.AluOpType
AF = mybir.ActivationFunctionType

# Problem constants
B = 16
NCORES = 8
G = B // NCORES          # graphs per core
N = 512                  # atoms per graph
L = 1024                 # amino length
CD = 128                 # comp_dim
PD = 128                 # prot_dim
GD = 64                  # gat_dim
H = 4                    # heads
LAT = 128                # latent
NA = 100                 # num_atom
NAM = 30                 # num_amino
LC = 3                   # conv layers
KW = 11                  # conv kernel width
ALPHA = 0.2
MASKNEG = -1.0e30
NT = N // 128            # 4 j-chunks


def build_core_program(debug=False):
    """Build the per-core SPMD program (identical across cores)."""
    if debug:
        nc = bacc.Bacc(None, target_bir_lowering=False, debug=True)
    else:
        nc = bacc.Bacc(None)

    # ---- DRAM I/O ----
    d_atoms = nc.dram_tensor("atoms_f", [G, N], F32, kind="ExternalInput")
    d_amask = nc.dram_tensor("atoms_mask", [G, N], F32, kind="ExternalInput")
    d_ladjT = nc.dram_tensor("ladjT", [G, N, N], F32, kind="ExternalInput")
    d_amino = nc.dram_tensor("amino_f", [G, L], F32, kind="ExternalInput")
    d_pmask = nc.dram_tensor("amino_mask", [G, L], F32, kind="ExternalInput")
    d_Eat = nc.dram_tensor("E_atom_pad", [128, CD], F32, kind="ExternalInput")
    d_Eam = nc.dram_tensor("E_amino", [NAM, PD], F32, kind="ExternalInput")
    d_Wg = nc.dram_tensor("W_gat", [H, CD, GD], F32, kind="ExternalInput")
    d_a1c = nc.dram_tensor("a1_cols", [GD, H], F32, kind="ExternalInput")
    d_a2c = nc.dram_tensor("a2_cols", [GD, H], F32, kind="ExternalInput")
    d_Wgo = nc.dram_tensor("W_go", [H * GD, CD], F32, kind="ExternalInput")
    d_ago = nc.dram_tensor("a_go_cols", [CD, 2], F32, kind="ExternalInput")
    d_Wc = nc.dram_tensor("W_comp_wT", [CD, LAT], F32, kind="ExternalInput")
    d_bc = nc.dram_tensor("W_comp_b", [LAT, 1], F32, kind="ExternalInput")
    d_MiT = nc.dram_tensor("MiT", [LC, KW, PD, PD], F32, kind="ExternalInput")
    d_cb = nc.dram_tensor("conv_b", [LC, 1], F32, kind="ExternalInput")
    d_Wa = nc.dram_tensor("W_att_wT", [LAT, LAT], F32, kind="ExternalInput")
    d_ba = nc.dram_tensor("W_att_b", [LAT, 1], F32, kind="ExternalInput")
    d_pw = nc.dram_tensor("pw_cols", [LAT, 2], F32, kind="ExternalInput")
    d_pb = nc.dram_tensor("pred_b", [1, 1], F32, kind="ExternalInput")
    d_out = nc.dram_tensor("out", [G, 1], F32, kind="ExternalOutput")

    with tile.TileContext(nc) as tc:
        with (
            tc.tile_pool(name="const", bufs=1) as cpool,
            tc.tile_pool(name="work", bufs=1) as wpool,
            tc.tile_pool(name="big", bufs=1) as bpool,
            tc.tile_pool(name="adj", bufs=2) as apool,
            tc.tile_pool(name="rows", bufs=1) as rpool,
            tc.tile_pool(name="ps_z", bufs=2, space="PSUM") as psz,
            tc.tile_pool(name="ps_sq", bufs=2, space="PSUM") as pssq,
            tc.tile_pool(name="ps_row", bufs=1, space="PSUM") as psrow,
            tc.tile_pool(name="ps_cv", bufs=2, space="PSUM") as pscv,
        ):
            # ---- constants / weights resident in SBUF ----
            ioi = cpool.tile([128, L], I32)
            nc.gpsimd.iota(ioi, pattern=[[0, L]], base=0, channel_multiplier=1)
            iof = cpool.tile([128, L], F32)
            nc.vector.tensor_copy(iof, ioi)
            ones_row = cpool.tile([1, L], F32)
            nc.vector.memset(ones_row, 1.0)
            ones_col = cpool.tile([128, 1], F32)
            nc.vector.memset(ones_col, 1.0)

            Eat = cpool.tile([128, CD], F32)
            nc.sync.dma_start(out=Eat, in_=d_Eat[:, :])
            Eam = cpool.tile([NAM, PD], F32)
            nc.sync.dma_start(out=Eam, in_=d_Eam[:, :])
            Wg = cpool.tile([CD, H, GD], F32)
            nc.sync.dma_start(out=Wg, in_=d_Wg.rearrange("h p q -> p h q"))
            a1c = cpool.tile([GD, H], F32)
            nc.sync.dma_start(out=a1c, in_=d_a1c[:, :])
            a2c = cpool.tile([GD, H], F32)
            nc.sync.dma_start(out=a2c, in_=d_a2c[:, :])
            Wgo = cpool.tile([128, 2, CD], F32)
            nc.sync.dma_start(out=Wgo, in_=d_Wgo.rearrange("(c p) q -> p c q", p=128))
            ago = cpool.tile([CD, 2], F32)
            nc.sync.dma_start(out=ago, in_=d_ago[:, :])
            Wc = cpool.tile([CD, LAT], F32)
            nc.sync.dma_start(out=Wc, in_=d_Wc[:, :])
            bc = cpool.tile([LAT, 1], F32)
            nc.sync.dma_start(out=bc, in_=d_bc[:, :])
            MiT = cpool.tile([PD, LC, KW, PD], F32)
            nc.sync.dma_start(out=MiT, in_=d_MiT.rearrange("l i p q -> p l i q"))
            cb = cpool.tile([128, LC], F32)
            # broadcast conv biases down all partitions
            nc.sync.dma_start(
                out=cb,
                in_=bass.AP(tensor=d_cb, offset=0, ap=[[0, 128], [1, LC], [0, 1]]),
            )
            Wa = cpool.tile([LAT, LAT], F32)
            nc.sync.dma_start(out=Wa, in_=d_Wa[:, :])
            ba = cpool.tile([LAT, 1], F32)
            nc.sync.dma_start(out=ba, in_=d_ba[:, :])
            pw = cpool.tile([LAT, 2], F32)
            nc.sync.dma_start(out=pw, in_=d_pw[:, :])
            pb = cpool.tile([1, 1], F32)
            nc.sync.dma_start(out=pb, in_=d_pb[:, :])

            for g in range(G):
                # ---------- atom embeddings via one-hot matmul ----------
                arow = rpool.tile([1, N], F32, tag="arow")
                nc.sync.dma_start(out=arow, in_=d_atoms[g : g + 1, :])
                ab = wpool.tile([128, N], F32, tag="ab")
                nc.gpsimd.partition_broadcast(ab, arow)
                ohA = wpool.tile([128, N], F32, tag="ohA")
                nc.vector.tensor_tensor(out=ohA, in0=ab, in1=iof[:, :N],
                                        op=AT.is_equal)
                avT_ps = pssq.tile([128, N], F32, tag="mm_sq")
                nc.tensor.matmul(avT_ps, Eat, ohA, start=True, stop=True)
                avT = wpool.tile([128, N], F32, tag="avT")
                nc.scalar.copy(avT, avT_ps)

                # ---------- ladjT for this graph ----------
                ladjT = apool.tile([128, NT, N], F32, tag="ladjT")
                nc.sync.dma_start(
                    out=ladjT,
                    in_=d_ladjT[g].rearrange("(t p) i -> p t i", p=128),
                )

                m01 = wpool.tile([128, N], F32, tag="multi01", name="m01")
                m23 = wpool.tile([128, N], F32, tag="multi23", name="m23")
                multi = [m01, m23]

                def gat_attention(whsb, nk, src_sb, dst_sb, hp_m, rowsum_sep,
                                  tag_pfx):
                    """Shared attention block: z -> leaky -> exp -> hp (+rowsum).

                    whsb: lhsT tiles (128, NT, nk[+1 ones col]) in SBUF
                    src_sb/dst_sb: (1, N) rows in SBUF
                    hp_m: output partitions (M) of the hp matmul
                    rowsum_sep: if True compute rowsum via separate ones-col
                        matmuls, else assume whsb carries a ones column and
                        rowsum lands in hp row nk.
                    Returns (hp_ps, zrow_src_ap)
                    """
                    zm = bpool.tile([128, NT, N], F32, tag="zm")
                    for t in range(NT):
                        zps = psz.tile([128, N], F32, tag="zps")
                        nc.tensor.matmul(
                            zps, dst_sb[:, t * 128 : (t + 1) * 128],
                            ones_row[:, :N], start=True, stop=False)
                        nc.tensor.matmul(
                            zps, ones_row[:, :128], src_sb,
                            start=False, stop=True)
                        nc.vector.scalar_tensor_tensor(
                            out=zm[:, t, :], in0=zps, scalar=1.0,
                            in1=ladjT[:, t, :], op0=AT.mult, op1=AT.add)
                    ee = bpool.tile([128, NT, N], F32, tag="ee")
                    nc.vector.scalar_tensor_tensor(
                        out=ee, in0=zm, scalar=ALPHA, in1=zm,
                        op0=AT.mult, op1=AT.max)
                    U = bpool.tile([128, NT, N], F32, tag="U")
                    nc.scalar.activation(out=U, in_=ee, func=AF.Exp)
                    hp_ps = pssq.tile([hp_m, N], F32, tag="mm_sq")
                    for t in range(NT):
                        nc.tensor.matmul(hp_ps, whsb[:, t, :], U[:, t, :],
                                         start=(t == 0), stop=(t == NT - 1))
                    if rowsum_sep:
                        rs_ps = psrow.tile([1, N], F32, tag="ps_row")
                        for t in range(NT):
                            nc.tensor.matmul(rs_ps, ones_col, U[:, t, :],
                                             start=(t == 0), stop=(t == NT - 1))
                        zrow = rs_ps
                    else:
                        zrow = hp_ps[nk : nk + 1, :]
                    # reciprocal of row, broadcast down partitions
                    zr = rpool.tile([1, N], F32, tag="zr")
                    nc.vector.tensor_copy(zr, zrow)
                    rinv = rpool.tile([1, N], F32, tag="rinv")
                    scr = rpool.tile([1, N], F32, tag="rscr")
                    nc.vector.reciprocal_approx_accurate(out=rinv, in_=zr,
                                                         scratch=scr)
                    rb = wpool.tile([nk, N], F32,
                                    tag="rb" + tag_pfx)
                    nc.gpsimd.partition_broadcast(rb, rinv)
                    return hp_ps, rb

                # ---------- GAT heads ----------
                for h in range(H):
                    whT_ps = pssq.tile([GD, N], F32, tag="mm_sq")
                    nc.tensor.matmul(whT_ps, Wg[:, h, :], avT,
                                     start=True, stop=True)
                    whT = wpool.tile([GD, N], F32, tag="whT")
                    nc.scalar.copy(whT, whT_ps)

                    wh_ps = pssq.tile([128, NT, GD], F32, tag="mm_sq")
                    for t in range(NT):
                        nc.tensor.matmul(
                            wh_ps[:, t, :], avT[:, t * 128 : (t + 1) * 128],
                            Wg[:, h, :], start=True, stop=True)
                    whsb = wpool.tile([128, NT, GD + 1], F32, tag="whsb")
                    nc.vector.memset(whsb, 1.0)
                    nc.scalar.copy(whsb[:, :, :GD], wh_ps)

                    src_ps = psrow.tile([1, N], F32, tag="ps_row")
                    nc.tensor.matmul(src_ps, a1c[:, h : h + 1], whT,
                                     start=True, stop=True)
                    src_sb = rpool.tile([1, N], F32, tag="src")
                    nc.vector.tensor_copy(src_sb, src_ps)
                    dst_ps = psrow.tile([1, N], F32, tag="ps_row")
                    nc.tensor.matmul(dst_ps, a2c[:, h : h + 1], whT,
                                     start=True, stop=True)
                    dst_sb = rpool.tile([1, N], F32, tag="dst")
                    nc.vector.tensor_copy(dst_sb, dst_ps)

                    hp_ps, rb = gat_attention(whsb, GD, src_sb, dst_sb,
                                              GD + 1, False, "h")
                    # normalize + ELU, write into multi tile half
                    hpn = wpool.tile([GD, N], F32, tag="hpn")
                    nc.vector.scalar_tensor_tensor(
                        out=hpn, in0=hp_ps[:GD, :], scalar=1.0, in1=rb,
                        op0=AT.mult, op1=AT.mult)
                    xm = wpool.tile([GD, N], F32, tag="xm")
                    nc.vector.tensor_scalar(out=xm, in0=hpn, scalar1=0.0,
                                            scalar2=None, op0=AT.min)
                    em = wpool.tile([GD, N], F32, tag="em")
                    nc.scalar.activation(out=em, in_=xm, func=AF.Exp)
                    dsti = multi[h // 2]
                    off = (h % 2) * GD
                    nc.vector.scalar_tensor_tensor(
                        out=dsti[off : off + GD, :], in0=em, scalar=-1.0,
                        in1=hpn, op0=AT.add, op1=AT.max)

                # ---------- GAT output layer ----------
                wh2T_ps = pssq.tile([CD, N], F32, tag="mm_sq")
                for c in range(2):
                    nc.tensor.matmul(wh2T_ps, Wgo[:, c, :], multi[c],
                                     start=(c == 0), stop=(c == 1))
                wh2T = wpool.tile([CD, N], F32, tag="whT")
                nc.scalar.copy(wh2T, wh2T_ps)

                wh2_ps = pssq.tile([128, NT, CD], F32, tag="mm_sq")
                for t in range(NT):
                    for c in range(2):
                        nc.tensor.matmul(
                            wh2_ps[:, t, :],
                            multi[c][:, t * 128 : (t + 1) * 128],
                            Wgo[:, c, :], start=(c == 0), stop=(c == 1))
                wh2sb = wpool.tile([128, NT, CD], F32, tag="wh2sb")
                nc.scalar.copy(wh2sb, wh2_ps)

                src2_ps = psrow.tile([1, N], F32, tag="ps_row")
                nc.tensor.matmul(src2_ps, ago[:, 0:1], wh2T,
                                 start=True, stop=True)
                src2 = rpool.tile([1, N], F32, tag="src")
                nc.vector.tensor_copy(src2, src2_ps)
                dst2_ps = psrow.tile([1, N], F32, tag="ps_row")
                nc.tensor.matmul(dst2_ps, ago[:, 1:2], wh2T,
                                 start=True, stop=True)
                dst2 = rpool.tile([1, N], F32, tag="dst")
                nc.vector.tensor_copy(dst2, dst2_ps)

                hp2_ps, rb2 = gat_attention(wh2sb, CD, src2, dst2,
                                            CD, True, "o")
                hpn2 = wpool.tile([CD, N], F32, tag="hpn2")
                nc.vector.scalar_tensor_tensor(
                    out=hpn2, in0=hp2_ps, scalar=1.0, in1=rb2,
                    op0=AT.mult, op1=AT.mult)
                xm2 = wpool.tile([CD, N], F32, tag="xm2")
                nc.vector.tensor_scalar(out=xm2, in0=hpn2, scalar1=0.0,
                                        scalar2=None, op0=AT.min)
                em2 = wpool.tile([CD, N], F32, tag="em2")
                nc.scalar.activation(out=em2, in_=xm2, func=AF.Exp)
                xT = wpool.tile([CD, N], F32, tag="xT")
                nc.vector.scalar_tensor_tensor(
                    out=xT, in0=em2, scalar=-1.0, in1=hpn2,
                    op0=AT.add, op1=AT.max)

                # ---------- atoms_vec -> a_v ----------
                av_ps = pssq.tile([LAT, N], F32, tag="mm_sq")
                nc.tensor.matmul(av_ps, Wc, xT, start=True, stop=True)
                avb = wpool.tile([LAT, N], F32, tag="avb")
                nc.scalar.activation(out=avb, in_=av_ps, func=AF.Identity,
                                     bias=bc)
                avec = wpool.tile([LAT, N], F32, tag="avec")
                nc.vector.scalar_tensor_tensor(
                    out=avec, in0=avb, scalar=ALPHA, in1=avb,
                    op0=AT.mult, op1=AT.max)
                av2_ps = pssq.tile([LAT, N], F32, tag="mm_sq")
                nc.tensor.matmul(av2_ps, Wa, avec, start=True, stop=True)
                avb2 = wpool.tile([LAT, N], F32, tag="avb2")
                nc.scalar.activation(out=avb2, in_=av2_ps, func=AF.Identity,
                                     bias=ba)
                a_v = wpool.tile([LAT, N], F32, tag="a_v")
                nc.vector.scalar_tensor_tensor(
                    out=a_v, in0=avb2, scalar=ALPHA, in1=avb2,
                    op0=AT.mult, op1=AT.max)

                # ---------- comp pooling ----------
                amrow = rpool.tile([1, N], F32, tag="amrow")
                nc.sync.dma_start(out=amrow, in_=d_amask[g : g + 1, :])
                amb = wpool.tile([128, N], F32, tag="amb")
                nc.gpsimd.partition_broadcast(amb, amrow)
                cscr = wpool.tile([LAT, N], F32, tag="cscr")
                comp_acc = rpool.tile([LAT, 1], F32, tag="comp_acc")
                nc.vector.scalar_tensor_tensor(
                    out=cscr, in0=a_v, scalar=1.0, in1=amb,
                    op0=AT.mult, op1=AT.mult, accum_out=comp_acc)
                amscr = rpool.tile([1, N], F32, tag="amscr")
                amsum = rpool.tile([1, 1], F32, tag="amsum")
                nc.vector.tensor_scalar(out=amscr, in0=amrow, scalar1=1.0,
                                        scalar2=0.0, op0=AT.mult, op1=AT.add,
                                        accum_out=amsum)
                amsb = rpool.tile([128, 1], F32, tag="amsb")
                nc.gpsimd.partition_broadcast(amsb, amsum)
                amr = rpool.tile([128, 1], F32, tag="amr")
                nc.vector.reciprocal(amr, amsb)
                cp = rpool.tile([128, 2], F32, tag="cp")
                nc.vector.tensor_scalar(out=cp[:, 0:1], in0=comp_acc,
                                        scalar1=amr, scalar2=None,
                                        op0=AT.mult)

                # ---------- protein CNN ----------
                prow = rpool.tile([1, L], F32, tag="prow")
                nc.sync.dma_start(out=prow, in_=d_amino[g : g + 1, :])
                pbm = wpool.tile([128, L], F32, tag="pbm")
                nc.gpsimd.partition_broadcast(pbm, prow)
                ohP = wpool.tile([NAM, L], F32, tag="ohP")
                nc.vector.tensor_tensor(out=ohP, in0=pbm[:NAM, :],
                                        in1=iof[:NAM, :], op=AT.is_equal)
                PADL = KW // 2
                pv = bpool.tile([PD, L + 2 * PADL], F32, tag="pv0")
                nc.vector.memset(pv[:, :PADL], 0.0)
                nc.vector.memset(pv[:, PADL + L :], 0.0)
                for nn in range(2):
                    pvT_ps = pscv.tile([PD, 512], F32, tag="ps_cv")
                    nc.tensor.matmul(pvT_ps, Eam,
                                     ohP[:, nn * 512 : (nn + 1) * 512],
                                     start=True, stop=True)
                    nc.scalar.copy(
                        pv[:, PADL + nn * 512 : PADL + (nn + 1) * 512], pvT_ps)
                for lyr in range(LC):
                    pvo = bpool.tile([PD, L + 2 * PADL], F32,
                                     tag=f"pv{1 - lyr % 2}")
                    nc.vector.memset(pvo[:, :PADL], 0.0)
                    nc.vector.memset(pvo[:, PADL + L :], 0.0)
                    for nn in range(2):
                        cv_ps = pscv.tile([PD, 512], F32, tag="ps_cv")
                        for i in range(KW):
                            nc.tensor.matmul(
                                cv_ps, MiT[:, lyr, i, :],
                                pv[:, nn * 512 + i : nn * 512 + i + 512],
                                start=(i == 0), stop=(i == KW - 1))
                        nc.scalar.activation(
                            out=pvo[:, PADL + nn * 512 : PADL + (nn + 1) * 512],
                            in_=cv_ps, func=AF.Relu, bias=cb[:, lyr : lyr + 1])
                    pv = pvo
                amv = pv[:, PADL : PADL + L]

                # ---------- p_v + prot pooling ----------
                p_v = wpool.tile([LAT, L], F32, tag="p_v")
                for nn in range(2):
                    pv_ps = pscv.tile([LAT, 512], F32, tag="ps_cv")
                    nc.tensor.matmul(pv_ps, Wa,
                                     amv[:, nn * 512 : (nn + 1) * 512],
                                     start=True, stop=True)
                    pvb = wpool.tile([LAT, 512], F32, tag="pvb")
                    nc.scalar.activation(out=pvb, in_=pv_ps,
                                         func=AF.Identity, bias=ba)
                    nc.vector.scalar_tensor_tensor(
                        out=p_v[:, nn * 512 : (nn + 1) * 512], in0=pvb,
                        scalar=ALPHA, in1=pvb, op0=AT.mult, op1=AT.max)
                pmrow = rpool.tile([1, L], F32, tag="pmrow")
                nc.sync.dma_start(out=pmrow, in_=d_pmask[g : g + 1, :])
                pmb = wpool.tile([128, L], F32, tag="pbm")
                nc.gpsimd.partition_broadcast(pmb, pmrow)
                pscr = wpool.tile([LAT, L], F32, tag="pscr")
                prot_acc = rpool.tile([LAT, 1], F32, tag="prot_acc")
                nc.vector.scalar_tensor_tensor(
                    out=pscr, in0=p_v, scalar=1.0, in1=pmb,
                    op0=AT.mult, op1=AT.mult, accum_out=prot_acc)
                pmscr = rpool.tile([1, L], F32, tag="pmscr")
                pmsum = rpool.tile([1, 1], F32, tag="pmsum")
                nc.vector.tensor_scalar(out=pmscr, in0=pmrow, scalar1=1.0,
                                        scalar2=0.0, op0=AT.mult, op1=AT.add,
                                        accum_out=pmsum)
                pmsb = rpool.tile([128, 1], F32, tag="pmsb")
                nc.gpsimd.partition_broadcast(pmsb, pmsum)
                pmr = rpool.tile([128, 1], F32, tag="pmr")
                nc.vector.reciprocal(pmr, pmsb)
                nc.vector.tensor_scalar(out=cp[:, 1:2], in0=prot_acc,
                                        scalar1=pmr, scalar2=None,
                                        op0=AT.mult)

                # ---------- prediction head ----------
                lr2 = rpool.tile([128, 2], F32, tag="lr2")
                nc.vector.scalar_tensor_tensor(
                    out=lr2, in0=cp, scalar=ALPHA * ALPHA, in1=cp,
                    op0=AT.mult, op1=AT.max)
                dscr = rpool.tile([128, 2], F32, tag="dscr")
                dacc = rpool.tile([128, 1], F32, tag="dacc")
                nc.vector.scalar_tensor_tensor(
                    out=dscr, in0=lr2, scalar=1.0, in1=pw,
                    op0=AT.mult, op1=AT.mult, accum_out=dacc)
                fin_ps = psrow.tile([1, 1], F32, tag="ps_fin")
                nc.tensor.matmul(fin_ps, dacc, ones_col, start=True, stop=True)
                res = rpool.tile([1, 1], F32, tag="res")
                nc.scalar.activation(out=res, in_=fin_ps, func=AF.Identity,
                                     bias=pb)
                nc.sync.dma_start(out=d_out[g : g + 1, :], in_=res)

    return nc


def preprocess(inputs):
    """Host-side prep: shard over cores, transpose/reshape weights."""
    atoms = np.asarray(inputs["atoms"]).astype(np.float32)
    atoms_mask = np.asarray(inputs["atoms_mask"]).astype(np.float32)
    adjacency = np.asarray(inputs["adjacency"])
    amino = np.asarray(inputs["amino"]).astype(np.float32)
    amino_mask = np.asarray(inputs["amino_mask"]).astype(np.float32)
    E_atom = np.asarray(inputs["E_atom"]).astype(np.float32)
    E_amino = np.asarray(inputs["E_amino"]).astype(np.float32)
    W_gat = np.asarray(inputs["W_gat"]).astype(np.float32)
    a_gat = np.asarray(inputs["a_gat"]).astype(np.float32)
    W_go = np.asarray(inputs["W_go"]).astype(np.float32)
    a_go = np.asarray(inputs["a_go"]).astype(np.float32)
    W_comp_w = np.asarray(inputs["W_comp_w"]).astype(np.float32)
    W_comp_b = np.asarray(inputs["W_comp_b"]).astype(np.float32)
    conv_w = np.asarray(inputs["conv_w"]).astype(np.float32)
    conv_b = np.asarray(inputs["conv_b"]).astype(np.float32)
    W_att_w = np.asarray(inputs["W_att_w"]).astype(np.float32)
    W_att_b = np.asarray(inputs["W_att_b"]).astype(np.float32)
    pred_w = np.asarray(inputs["pred_w"]).astype(np.float32)
    pred_b = np.asarray(inputs["pred_b"]).astype(np.float32)

    # masked-adjacency additive term, transposed: ladjT[g, j, i]
    ladjT = np.where(adjacency.transpose(0, 2, 1) > 0, np.float32(0.0),
                     np.float32(MASKNEG)).astype(np.float32)

    E_atom_pad = np.zeros((128, CD), np.float32)
    E_atom_pad[:NA] = E_atom

    # conv band matrices: MiT[l, i, din, dout] = conv_w[l,0,0,i, din-dout+5]
    MiT = np.zeros((LC, KW, PD, PD), np.float32)
    din = np.arange(PD)[:, None]
    dout = np.arange(PD)[None, :]
    v = din - dout + (KW // 2)
    valid = (v >= 0) & (v < KW)
    vc = np.clip(v, 0, KW - 1)
    for lyr in range(LC):
        for i in range(KW):
            MiT[lyr, i] = np.where(valid, conv_w[lyr, 0, 0, i, vc], 0.0)

    shared = {
        "E_atom_pad": E_atom_pad,
        "E_amino": np.ascontiguousarray(E_amino),
        "W_gat": np.ascontiguousarray(W_gat),
        "a1_cols": np.ascontiguousarray(a_gat[:, :GD].T),
        "a2_cols": np.ascontiguousarray(a_gat[:, GD:].T),
        "W_go": np.ascontiguousarray(W_go),
        "a_go_cols": np.ascontiguousarray(
            np.stack([a_go[:CD], a_go[CD:]], axis=1)),
        "W_comp_wT": np.ascontiguousarray(W_comp_w.T),
        "W_comp_b": np.ascontiguousarray(W_comp_b[:, None]),
        "MiT": MiT,
        "conv_b": np.ascontiguousarray(conv_b.reshape(LC, 1)),
        "W_att_wT": np.ascontiguousarray(W_att_w.T),
        "W_att_b": np.ascontiguousarray(W_att_b[:, None]),
        "pw_cols": np.ascontiguousarray(
            np.stack([pred_w[0, :LAT], pred_w[0, LAT:]], axis=1)),
        "pred_b": np.ascontiguousarray(pred_b.reshape(1, 1)),
    }
    in_maps = []
    for c in range(NCORES):
        sl = slice(c * G, (c + 1) * G)
        m = dict(shared)
        m["atoms_f"] = np.ascontiguousarray(atoms[sl])
        m["atoms_mask"] = np.ascontiguousarray(atoms_mask[sl])
        m["ladjT"] = np.ascontiguousarray(ladjT[sl])
        m["amino_f"] = np.ascontiguousarray(amino[sl])
        m["amino_mask"] = np.ascontiguousarray(amino_mask[sl])
        in_maps.append(m)
    return in_maps


_CACHED_NC = None


def kernel(**inputs) -> np.ndarray:
    global _CACHED_NC
    from concourse.bass_utils import run_bass_kernel_spmd

    if _CACHED_NC is None:
        nc = build_core_program()
        nc.finalize()
        _CACHED_NC = nc
    nc = _CACHED_NC
    in_maps = preprocess(inputs)
    res = run_bass_kernel_spmd(nc, in_maps, core_ids=list(range(NCORES)))
    out = np.concatenate([res.results[c]["out"] for c in range(NCORES)], axis=0)
    return out.astype(np.float32)


# revision 10
# speedup vs baseline: 1.3175x; 1.3175x over previous
"""Trainium2 Bass kernel for BiDACPI (GAT + CNN + bidirectional attention).

Data-parallel over batch b=16 across 8 NeuronCores (2 graphs per core).
Self-contained: hardcodes all shapes; host-side preprocessing only reshapes /
transposes weights and converts index tensors.
"""
import numpy as np

import concourse.bass as bass
import concourse.mybir as mybir
import concourse.tile as tile
from concourse import bacc

F32 = mybir.dt.float32
I32 = mybir.dt.int32
AT = mybir.AluOpType
AF = mybir.ActivationFunctionType

# Problem constants
B = 16
NCORES = 8
G = B // NCORES          # graphs per core
N = 512                  # atoms per graph
L = 1024                 # amino length
CD = 128                 # comp_dim
PD = 128                 # prot_dim
GD = 64                  # gat_dim
H = 4                    # heads
LAT = 128                # latent
NA = 100                 # num_atom
NAM = 30                 # num_amino
LC = 3                   # conv layers
KW = 11                  # conv kernel width
ALPHA = 0.2
MASKNEG = -1.0e30
NT = N // 128            # 4 j-chunks


def build_core_program(debug=False):
    """Build the per-core SPMD program (identical across cores).

    debug=True builds the CoreSim-compatible variant (no Prelu — the sim
    lacks it; uses the DVE max(ax, x) leaky instead).
    """
    if debug:
        nc = bacc.Bacc(None, target_bir_lowering=False, debug=True)
    else:
        nc = bacc.Bacc(None)
    use_prelu = not debug

    # ---- DRAM I/O ----
    d_atoms = nc.dram_tensor("atoms_f", [G, N], F32, kind="ExternalInput")
    d_amask = nc.dram_tensor("atoms_mask", [G, N], F32, kind="ExternalInput")
    d_ladjT = nc.dram_tensor("ladjT", [G, N, N], F32, kind="ExternalInput")
    d_amino = nc.dram_tensor("amino_f", [G, L], F32, kind="ExternalInput")
    d_pmask = nc.dram_tensor("amino_mask", [G, L], F32, kind="ExternalInput")
    d_Eat = nc.dram_tensor("E_atom_pad", [128, CD], F32, kind="ExternalInput")
    d_Eam = nc.dram_tensor("E_amino", [NAM, PD], F32, kind="ExternalInput")
    d_Wg = nc.dram_tensor("W_gat", [H, CD, GD], F32, kind="ExternalInput")
    d_a12 = nc.dram_tensor("a12_cols", [GD, H, 2], F32, kind="ExternalInput")
    d_Wgo = nc.dram_tensor("W_go", [H * GD, CD], F32, kind="ExternalInput")
    d_ago = nc.dram_tensor("a_go_cols", [CD, 2], F32, kind="ExternalInput")
    d_Wc = nc.dram_tensor("W_comp_wT", [CD, LAT], F32, kind="ExternalInput")
    d_bc = nc.dram_tensor("W_comp_b", [LAT, 1], F32, kind="ExternalInput")
    d_MiT = nc.dram_tensor("MiT", [LC, KW, PD, PD], F32, kind="ExternalInput")
    d_cb = nc.dram_tensor("conv_b", [LC, 1], F32, kind="ExternalInput")
    d_Wa = nc.dram_tensor("W_att_wT", [LAT, LAT], F32, kind="ExternalInput")
    d_ba = nc.dram_tensor("W_att_b", [LAT, 1], F32, kind="ExternalInput")
    d_pw = nc.dram_tensor("pw_cols", [LAT, 2], F32, kind="ExternalInput")
    d_pb = nc.dram_tensor("pred_b", [1, 1], F32, kind="ExternalInput")
    d_out = nc.dram_tensor("out", [G, 1], F32, kind="ExternalOutput")

    with tile.TileContext(nc) as tc:
        with (
            tc.tile_pool(name="const", bufs=1) as cpool,
            tc.tile_pool(name="work", bufs=1) as wpool,
            tc.tile_pool(name="big", bufs=2) as bpool,
            tc.tile_pool(name="adj", bufs=2) as apool,
            tc.tile_pool(name="rows", bufs=1) as rpool,
            tc.tile_pool(name="ps_z", bufs=2, space="PSUM") as psz,
            tc.tile_pool(name="ps_sq", bufs=3, space="PSUM") as pssq,
            tc.tile_pool(name="ps_row", bufs=1, space="PSUM") as psrow,
            tc.tile_pool(name="ps_cv", bufs=2, space="PSUM") as pscv,
        ):
            # ---- constants / weights resident in SBUF ----
            ioi = cpool.tile([128, L], I32)
            nc.gpsimd.iota(ioi, pattern=[[0, L]], base=0, channel_multiplier=1)
            iof = cpool.tile([128, L], F32)
            nc.vector.tensor_copy(iof, ioi)
            ones_row = cpool.tile([1, L], F32)
            nc.vector.memset(ones_row, 1.0)
            ones_col = cpool.tile([128, 1], F32)
            nc.vector.memset(ones_col, 1.0)

            Eat = cpool.tile([128, CD], F32)
            nc.sync.dma_start(out=Eat, in_=d_Eat[:, :])
            Eam = cpool.tile([NAM, PD], F32)
            nc.sync.dma_start(out=Eam, in_=d_Eam[:, :])
            Wg = cpool.tile([CD, H, GD], F32)
            nc.sync.dma_start(out=Wg, in_=d_Wg.rearrange("h p q -> p h q"))
            a12 = cpool.tile([GD, H, 2], F32)
            nc.sync.dma_start(out=a12, in_=d_a12[:, :, :])
            Wgo = cpool.tile([128, 2, CD], F32)
            nc.sync.dma_start(out=Wgo, in_=d_Wgo.rearrange("(c p) q -> p c q", p=128))
            ago = cpool.tile([CD, 2], F32)
            nc.sync.dma_start(out=ago, in_=d_ago[:, :])
            Wc = cpool.tile([CD, LAT], F32)
            nc.sync.dma_start(out=Wc, in_=d_Wc[:, :])
            bc = cpool.tile([LAT, 1], F32)
            nc.sync.dma_start(out=bc, in_=d_bc[:, :])
            MiT = cpool.tile([PD, LC, KW, PD], F32)
            nc.sync.dma_start(out=MiT, in_=d_MiT.rearrange("l i p q -> p l i q"))
            cb = cpool.tile([128, LC], F32)
            nc.sync.dma_start(
                out=cb,
                in_=bass.AP(tensor=d_cb, offset=0, ap=[[0, 128], [1, LC], [0, 1]]),
            )
            Wa = cpool.tile([LAT, LAT], F32)
            nc.sync.dma_start(out=Wa, in_=d_Wa[:, :])
            ba = cpool.tile([LAT, 1], F32)
            nc.sync.dma_start(out=ba, in_=d_ba[:, :])
            pw = cpool.tile([LAT, 2], F32)
            nc.sync.dma_start(out=pw, in_=d_pw[:, :])
            pb = cpool.tile([1, 1], F32)
            nc.sync.dma_start(out=pb, in_=d_pb[:, :])

            def leaky(out, in_, alpha, bias=None):
                """out = leaky_relu(in_ + bias, alpha). in_ may be PSUM."""
                if use_prelu:
                    if bias is None:
                        nc.scalar.activation(out=out, in_=in_, func=AF.Prelu,
                                             alpha=alpha)
                    else:
                        nc.scalar.activation(out=out, in_=in_, func=AF.Prelu,
                                             bias=bias, alpha=alpha)
                    return
                src = in_
                if bias is not None:
                    t = wpool.tile(list(out.shape), F32, tag="t2k",
                                   bufs=6, name="lkb")
                    nc.scalar.activation(out=t, in_=in_, func=AF.Identity,
                                         bias=bias)
                    src = t
                nc.vector.scalar_tensor_tensor(
                    out=out, in0=src, scalar=alpha, in1=src,
                    op0=AT.mult, op1=AT.max)

            def elu_into(out_ap, hp_src, rb, m, tag_sfx):
                """out = elu(hp_src * rb); hp_src PSUM (m, N), rb SBUF (m, N)."""
                hpn = wpool.tile([m, N], F32, tag="t2k", bufs=6,
                                 name="hpn" + tag_sfx)
                nc.vector.scalar_tensor_tensor(
                    out=hpn, in0=hp_src, scalar=1.0, in1=rb,
                    op0=AT.mult, op1=AT.mult)
                xm = wpool.tile([m, N], F32, tag="t2k", bufs=6,
                                name="xm" + tag_sfx)
                nc.vector.tensor_scalar(out=xm, in0=hpn, scalar1=0.0,
                                        scalar2=None, op0=AT.min)
                em = wpool.tile([m, N], F32, tag="t2k", bufs=6,
                                name="em" + tag_sfx)
                nc.scalar.activation(out=em, in_=xm, func=AF.Exp)
                nc.vector.scalar_tensor_tensor(
                    out=out_ap, in0=em, scalar=-1.0, in1=hpn,
                    op0=AT.add, op1=AT.max)

            for g in range(G):
                # ---------- atom embeddings via one-hot matmul ----------
                arow = rpool.tile([1, N], F32, tag="r2k", bufs=4, name="arow")
                nc.sync.dma_start(out=arow, in_=d_atoms[g : g + 1, :])
                ab = wpool.tile([128, N], F32, tag="t2k", bufs=6, name="ab")
                nc.gpsimd.partition_broadcast(ab, arow)
                ohA = wpool.tile([128, N], F32, tag="t2k", bufs=6, name="ohA")
                nc.vector.tensor_tensor(out=ohA, in0=ab, in1=iof[:, :N],
                                        op=AT.is_equal)
                avT_ps = pssq.tile([128, N], F32, tag="mm_sq", name="avT_ps")
                nc.tensor.matmul(avT_ps, Eat, ohA, start=True, stop=True)
                avT = wpool.tile([128, N], F32, tag="avT", bufs=2, name="avT")
                nc.scalar.copy(avT, avT_ps)

                # ---------- ladjT for this graph ----------
                ladjT = apool.tile([128, NT, N], F32, tag="ladjT", name="ladjT")
                nc.sync.dma_start(
                    out=ladjT,
                    in_=d_ladjT[g].rearrange("(t p) i -> p t i", p=128),
                )

                m01 = wpool.tile([128, N], F32, tag="multi01", bufs=2, name="m01")
                m23 = wpool.tile([128, N], F32, tag="multi23", bufs=2, name="m23")
                multi = [m01, m23]

                def gat_attention(whsb, nk, sd_ps, rowsum_sep, tag_pfx):
                    """z -> leaky -> exp -> hp (+rowsum) -> 1/rowsum bcast.

                    sd_ps: PSUM (2, N): row0 = src, row1 = dst.
                    Returns (hp_ps, rb).
                    """
                    # compound K=2 operands: zl = [dst; ones], zr = [ones; src]
                    sd = rpool.tile([2, N], F32, tag="sd", bufs=2, name="sd")
                    nc.vector.tensor_copy(sd, sd_ps)
                    zl = rpool.tile([2, N], F32, tag="zl", bufs=2, name="zl")
                    zr = rpool.tile([2, N], F32, tag="zr", bufs=2, name="zr")
                    nc.sync.dma_start(out=zl[0:1, :], in_=sd[1:2, :])
                    nc.sync.dma_start(out=zl[1:2, :], in_=ones_row[:, :N])
                    nc.vector.memset(zr[0:1, :], 1.0)
                    nc.sync.dma_start(out=zr[1:2, :], in_=sd[0:1, :])

                    zm = bpool.tile([128, NT, N], F32, tag="zm", name="zm")
                    for t in range(NT):
                        zps = psz.tile([128, N], F32, tag="zps", name="zps")
                        nc.tensor.matmul(zps, zl[:, t * 128 : (t + 1) * 128],
                                         zr, start=True, stop=True)
                        nc.vector.scalar_tensor_tensor(
                            out=zm[:, t, :], in0=zps, scalar=1.0,
                            in1=ladjT[:, t, :], op0=AT.mult, op1=AT.add)
                    ee = bpool.tile([128, NT, N], F32, tag="ee", name="ee")
                    leaky(ee, zm, ALPHA)
                    U = bpool.tile([128, NT, N], F32, tag="U", name="U")
                    nc.scalar.activation(out=U, in_=ee, func=AF.Exp)
                    hp_m = nk + (0 if rowsum_sep else 1)
                    hp_ps = pssq.tile([128, N], F32, tag="mm_sq", name="hp_ps")
                    for t in range(NT):
                        nc.tensor.matmul(hp_ps[:hp_m, :],
                                         whsb[:, t, :], U[:, t, :],
                                         start=(t == 0), stop=(t == NT - 1))
                    if rowsum_sep:
                        rs_ps = psrow.tile([1, N], F32, tag="ps_row",
                                           name="rs_ps")
                        for t in range(NT):
                            nc.tensor.matmul(rs_ps, ones_col, U[:, t, :],
                                             start=(t == 0), stop=(t == NT - 1))
                        zrow = rs_ps
                    else:
                        zrow = hp_ps[nk : nk + 1, :]
                    zrw = rpool.tile([1, N], F32, tag="r2k", bufs=4, name="zrw")
                    nc.vector.tensor_copy(zrw, zrow)
                    rinv = rpool.tile([1, N], F32, tag="r2k", bufs=4, name="rinv")
                    scr = rpool.tile([1, N], F32, tag="r2k", bufs=4, name="rscr")
                    nc.vector.reciprocal_approx_accurate(out=rinv, in_=zrw,
                                                         scratch=scr)
                    rb = wpool.tile([nk, N], F32, tag="t2k", bufs=6,
                                    name="rb" + tag_pfx)
                    nc.gpsimd.partition_broadcast(rb, rinv)
                    return hp_ps, rb

                # ---------- GAT heads ----------
                for h in range(H):
                    whT_ps = pssq.tile([GD, N], F32, tag="mm_sq", name="whT_ps")
                    nc.tensor.matmul(whT_ps, Wg[:, h, :], avT,
                                     start=True, stop=True)
                    whT = wpool.tile([GD, N], F32, tag="t2k", bufs=6, name="whT")
                    nc.scalar.copy(whT, whT_ps)

                    wh_ps = pssq.tile([128, NT, GD], F32, tag="mm_sq",
                                      name="wh_ps")
                    for t in range(NT):
                        nc.tensor.matmul(
                            wh_ps[:, t, :], avT[:, t * 128 : (t + 1) * 128],
                            Wg[:, h, :], start=True, stop=True)
                    whsb = wpool.tile([128, NT, GD + 1], F32, tag="t2k",
                                      bufs=6, name="whsb")
                    nc.vector.memset(whsb, 1.0)
                    nc.scalar.copy(whsb[:, :, :GD], wh_ps)

                    sd_ps = psrow.tile([2, N], F32, tag="ps_row", name="sd_ps")
                    nc.tensor.matmul(sd_ps, a12[:, h, :], whT,
                                     start=True, stop=True)

                    hp_ps, rb = gat_attention(whsb, GD, sd_ps, False, "h")
                    dsti = multi[h // 2]
                    off = (h % 2) * GD
                    elu_into(dsti[off : off + GD, :], hp_ps[:GD, :], rb,
                             GD, "h")

                # ---------- GAT output layer ----------
                wh2T_ps = pssq.tile([CD, N], F32, tag="mm_sq", name="wh2T_ps")
                for c in range(2):
                    nc.tensor.matmul(wh2T_ps, Wgo[:, c, :], multi[c],
                                     start=(c == 0), stop=(c == 1))
                wh2T = wpool.tile([CD, N], F32, tag="t2k", bufs=6, name="wh2T")
                nc.scalar.copy(wh2T, wh2T_ps)

                wh2_ps = pssq.tile([128, NT, CD], F32, tag="mm_sq",
                                   name="wh2_ps")
                for t in range(NT):
                    for c in range(2):
                        nc.tensor.matmul(
                            wh2_ps[:, t, :],
                            multi[c][:, t * 128 : (t + 1) * 128],
                            Wgo[:, c, :], start=(c == 0), stop=(c == 1))
                wh2sb = wpool.tile([128, NT, CD], F32, tag="t2k", bufs=6,
                                   name="wh2sb")
                nc.scalar.copy(wh2sb, wh2_ps)

                sd2_ps = psrow.tile([2, N], F32, tag="ps_row", name="sd2_ps")
                nc.tensor.matmul(sd2_ps, ago, wh2T, start=True, stop=True)

                hp2_ps, rb2 = gat_attention(wh2sb, CD, sd2_ps, True, "o")
                xT = wpool.tile([CD, N], F32, tag="xT", bufs=2, name="xT")
                elu_into(xT, hp2_ps, rb2, CD, "o")

                # ---------- atoms_vec -> a_v ----------
                av_ps = pssq.tile([LAT, N], F32, tag="mm_sq", name="av_ps")
                nc.tensor.matmul(av_ps, Wc, xT, start=True, stop=True)
                avec = wpool.tile([LAT, N], F32, tag="t2k", bufs=6, name="avec")
                leaky(avec, av_ps, ALPHA, bias=bc)
                av2_ps = pssq.tile([LAT, N], F32, tag="mm_sq", name="av2_ps")
                nc.tensor.matmul(av2_ps, Wa, avec, start=True, stop=True)
                a_v = wpool.tile([LAT, N], F32, tag="t2k", bufs=6, name="a_v")
                leaky(a_v, av2_ps, ALPHA, bias=ba)

                # ---------- comp pooling ----------
                amrow = rpool.tile([1, N], F32, tag="r2k", bufs=4, name="amrow")
                nc.sync.dma_start(out=amrow, in_=d_amask[g : g + 1, :])
                amb = wpool.tile([128, N], F32, tag="t2k", bufs=6, name="amb")
                nc.gpsimd.partition_broadcast(amb, amrow)
                cscr = wpool.tile([LAT, N], F32, tag="t2k", bufs=6, name="cscr")
                comp_acc = rpool.tile([LAT, 1], F32, tag="c1", bufs=2,
                                      name="comp_acc")
                nc.vector.scalar_tensor_tensor(
                    out=cscr, in0=a_v, scalar=1.0, in1=amb,
                    op0=AT.mult, op1=AT.mult, accum_out=comp_acc)
                amscr = rpool.tile([1, N], F32, tag="r2k", bufs=4, name="amscr")
                amsum = rpool.tile([1, 1], F32, tag="c2", bufs=4, name="amsum")
                nc.vector.tensor_scalar(out=amscr, in0=amrow, scalar1=1.0,
                                        scalar2=0.0, op0=AT.mult, op1=AT.add,
                                        accum_out=amsum)
                amsb = rpool.tile([128, 1], F32, tag="c2", bufs=4, name="amsb")
                nc.gpsimd.partition_broadcast(amsb, amsum)
                amr = rpool.tile([128, 1], F32, tag="c2", bufs=4, name="amr")
                nc.vector.reciprocal(amr, amsb)
                cp = rpool.tile([128, 2], F32, tag="cp", bufs=2, name="cp")
                nc.vector.tensor_scalar(out=cp[:, 0:1], in0=comp_acc,
                                        scalar1=amr, scalar2=None,
                                        op0=AT.mult)

                # ---------- protein CNN ----------
                prow = rpool.tile([1, L], F32, tag="r4k", bufs=2, name="prow")
                nc.sync.dma_start(out=prow, in_=d_amino[g : g + 1, :])
                pbm = wpool.tile([128, L], F32, tag="t4k", bufs=3, name="pbm")
                nc.gpsimd.partition_broadcast(pbm, prow)
                ohP = wpool.tile([NAM, L], F32, tag="t4k", bufs=3, name="ohP")
                nc.vector.tensor_tensor(out=ohP, in0=pbm[:NAM, :],
                                        in1=iof[:NAM, :], op=AT.is_equal)
                PADL = KW // 2
                pv = bpool.tile([PD, L + 2 * PADL], F32, tag="pv0", name="pv")
                nc.vector.memset(pv[:, :PADL], 0.0)
                nc.vector.memset(pv[:, PADL + L :], 0.0)
                for nn in range(2):
                    pvT_ps = pscv.tile([PD, 512], F32, tag="ps_cv",
                                       name="pvT_ps")
                    nc.tensor.matmul(pvT_ps, Eam,
                                     ohP[:, nn * 512 : (nn + 1) * 512],
                                     start=True, stop=True)
                    nc.scalar.copy(
                        pv[:, PADL + nn * 512 : PADL + (nn + 1) * 512], pvT_ps)
                for lyr in range(LC):
                    pvo = bpool.tile([PD, L + 2 * PADL], F32,
                                     tag=f"pv{1 - lyr % 2}", name="pvo")
                    nc.vector.memset(pvo[:, :PADL], 0.0)
                    nc.vector.memset(pvo[:, PADL + L :], 0.0)
                    for nn in range(2):
                        cv_ps = pscv.tile([PD, 512], F32, tag="ps_cv",
                                          name="cv_ps")
                        for i in range(KW):
                            nc.tensor.matmul(
                                cv_ps, MiT[:, lyr, i, :],
                                pv[:, nn * 512 + i : nn * 512 + i + 512],
                                start=(i == 0), stop=(i == KW - 1))
                        nc.scalar.activation(
                            out=pvo[:, PADL + nn * 512 : PADL + (nn + 1) * 512],
                            in_=cv_ps, func=AF.Relu, bias=cb[:, lyr : lyr + 1])
                    pv = pvo
                amv = pv[:, PADL : PADL + L]

                # ---------- p_v + prot pooling ----------
                p_v = wpool.tile([LAT, L], F32, tag="t4k", bufs=3, name="p_v")
                for nn in range(2):
                    pv_ps = pscv.tile([LAT, 512], F32, tag="ps_cv",
                                      name="pv_ps")
                    nc.tensor.matmul(pv_ps, Wa,
                                     amv[:, nn * 512 : (nn + 1) * 512],
                                     start=True, stop=True)
                    leaky(p_v[:, nn * 512 : (nn + 1) * 512], pv_ps, ALPHA,
                          bias=ba)
                pmrow = rpool.tile([1, L], F32, tag="r4k", bufs=2, name="pmrow")
                nc.sync.dma_start(out=pmrow, in_=d_pmask[g : g + 1, :])
                pmb = wpool.tile([128, L], F32, tag="t4k", bufs=3, name="pmb")
                nc.gpsimd.partition_broadcast(pmb, pmrow)
                pscr = wpool.tile([LAT, L], F32, tag="t4k", bufs=3, name="pscr")
                prot_acc = rpool.tile([LAT, 1], F32, tag="c1", bufs=2,
                                      name="prot_acc")
                nc.vector.scalar_tensor_tensor(
                    out=pscr, in0=p_v, scalar=1.0, in1=pmb,
                    op0=AT.mult, op1=AT.mult, accum_out=prot_acc)
                pmscr = rpool.tile([1, L], F32, tag="r4k", bufs=2, name="pmscr")
                pmsum = rpool.tile([1, 1], F32, tag="c2", bufs=4, name="pmsum")
                nc.vector.tensor_scalar(out=pmscr, in0=pmrow, scalar1=1.0,
                                        scalar2=0.0, op0=AT.mult, op1=AT.add,
                                        accum_out=pmsum)
                pmsb = rpool.tile([128, 1], F32, tag="c2", bufs=4, name="pmsb")
                nc.gpsimd.partition_broadcast(pmsb, pmsum)
                pmr = rpool.tile([128, 1], F32, tag="c2", bufs=4, name="pmr")
                nc.vector.reciprocal(pmr, pmsb)
                nc.vector.tensor_scalar(out=cp[:, 1:2], in0=prot_acc,
                                        scalar1=pmr, scalar2=None,
                                        op0=AT.mult)

                # ---------- prediction head ----------
                lr2 = rpool.tile([128, 2], F32, tag="cp", bufs=2, name="lr2")
                leaky(lr2, cp, ALPHA * ALPHA)
                dscr = rpool.tile([128, 2], F32, tag="cp", bufs=2, name="dscr")
                dacc = rpool.tile([128, 1], F32, tag="c1", bufs=2, name="dacc")
                nc.vector.scalar_tensor_tensor(
                    out=dscr, in0=lr2, scalar=1.0, in1=pw,
                    op0=AT.mult, op1=AT.mult, accum_out=dacc)
                fin_ps = psrow.tile([1, 1], F32, tag="ps_row", name="fin_ps")
                nc.tensor.matmul(fin_ps, dacc, ones_col, start=True, stop=True)
                res = rpool.tile([1, 1], F32, tag="c2", bufs=4, name="res")
                nc.scalar.activation(out=res, in_=fin_ps, func=AF.Identity,
                                     bias=pb)
                nc.sync.dma_start(out=d_out[g : g + 1, :], in_=res)

    return nc


def preprocess(inputs):
    """Host-side prep: shard over cores, transpose/reshape weights."""
    atoms = np.asarray(inputs["atoms"]).astype(np.float32)
    atoms_mask = np.asarray(inputs["atoms_mask"]).astype(np.float32)
    adjacency = np.asarray(inputs["adjacency"])
    amino = np.asarray(inputs["amino"]).astype(np.float32)
    amino_mask = np.asarray(inputs["amino_mask"]).astype(np.float32)
    E_atom = np.asarray(inputs["E_atom"]).astype(np.float32)
    E_amino = np.asarray(inputs["E_amino"]).astype(np.float32)
    W_gat = np.asarray(inputs["W_gat"]).astype(np.float32)
    a_gat = np.asarray(inputs["a_gat"]).astype(np.float32)
    W_go = np.asarray(inputs["W_go"]).astype(np.float32)
    a_go = np.asarray(inputs["a_go"]).astype(np.float32)
    W_comp_w = np.asarray(inputs["W_comp_w"]).astype(np.float32)
    W_comp_b = np.asarray(inputs["W_comp_b"]).astype(np.float32)
    conv_w = np.asarray(inputs["conv_w"]).astype(np.float32)
    conv_b = np.asarray(inputs["conv_b"]).astype(np.float32)
    W_att_w = np.asarray(inputs["W_att_w"]).astype(np.float32)
    W_att_b = np.asarray(inputs["W_att_b"]).astype(np.float32)
    pred_w = np.asarray(inputs["pred_w"]).astype(np.float32)
    pred_b = np.asarray(inputs["pred_b"]).astype(np.float32)

    ladjT = np.where(adjacency.transpose(0, 2, 1) > 0, np.float32(0.0),
                     np.float32(MASKNEG)).astype(np.float32)

    E_atom_pad = np.zeros((128, CD), np.float32)
    E_atom_pad[:NA] = E_atom

    # conv band matrices: MiT[l, i, din, dout] = conv_w[l,0,0,i, din-dout+5]
    MiT = np.zeros((LC, KW, PD, PD), np.float32)
    din = np.arange(PD)[:, None]
    dout = np.arange(PD)[None, :]
    v = din - dout + (KW // 2)
    valid = (v >= 0) & (v < KW)
    vc = np.clip(v, 0, KW - 1)
    for lyr in range(LC):
        for i in range(KW):
            MiT[lyr, i] = np.where(valid, conv_w[lyr, 0, 0, i, vc], 0.0)

    # a12_cols[f, h, 0] = src vec a1; [f, h, 1] = dst vec a2
    a12 = np.stack([a_gat[:, :GD].T, a_gat[:, GD:].T], axis=2)

    shared = {
        "E_atom_pad": E_atom_pad,
        "E_amino": np.ascontiguousarray(E_amino),
        "W_gat": np.ascontiguousarray(W_gat),
        "a12_cols": np.ascontiguousarray(a12),
        "W_go": np.ascontiguousarray(W_go),
        "a_go_cols": np.ascontiguousarray(
            np.stack([a_go[:CD], a_go[CD:]], axis=1)),
        "W_comp_wT": np.ascontiguousarray(W_comp_w.T),
        "W_comp_b": np.ascontiguousarray(W_comp_b[:, None]),
        "MiT": MiT,
        "conv_b": np.ascontiguousarray(conv_b.reshape(LC, 1)),
        "W_att_wT": np.ascontiguousarray(W_att_w.T),
        "W_att_b": np.ascontiguousarray(W_att_b[:, None]),
        "pw_cols": np.ascontiguousarray(
            np.stack([pred_w[0, :LAT], pred_w[0, LAT:]], axis=1)),
        "pred_b": np.ascontiguousarray(pred_b.reshape(1, 1)),
    }
    in_maps = []
    for c in range(NCORES):
        sl = slice(c * G, (c + 1) * G)
        m = dict(shared)
        m["atoms_f"] = np.ascontiguousarray(atoms[sl])
        m["atoms_mask"] = np.ascontiguousarray(atoms_mask[sl])
        m["ladjT"] = np.ascontiguousarray(ladjT[sl])
        m["amino_f"] = np.ascontiguousarray(amino[sl])
        m["amino_mask"] = np.ascontiguousarray(amino_mask[sl])
        in_maps.append(m)
    return in_maps


_CACHED_NC = None


def kernel(**inputs) -> np.ndarray:
    global _CACHED_NC
    from concourse.bass_utils import run_bass_kernel_spmd

    if _CACHED_NC is None:
        nc = build_core_program()
        nc.finalize()
        _CACHED_NC = nc
    nc = _CACHED_NC
    in_maps = preprocess(inputs)
    res = run_bass_kernel_spmd(nc, in_maps, core_ids=list(range(NCORES)))
    out = np.concatenate([res.results[c]["out"] for c in range(NCORES)], axis=0)
    return out.astype(np.float32)


# revision 13
# speedup vs baseline: 1.7634x; 1.3385x over previous
"""Trainium2 Bass kernel for BiDACPI (GAT + CNN + bidirectional attention).

Data-parallel over batch b=16 across 8 NeuronCores (2 graphs per core).
Self-contained: hardcodes all shapes; host-side preprocessing only reshapes /
transposes weights and converts index tensors.
"""
import numpy as np

import concourse.bass as bass
import concourse.mybir as mybir
import concourse.tile as tile
from concourse import bacc

F32 = mybir.dt.float32
I32 = mybir.dt.int32
AT = mybir.AluOpType
AF = mybir.ActivationFunctionType

# Problem constants
B = 16
NCORES = 8
G = B // NCORES          # graphs per core
N = 512                  # atoms per graph
L = 1024                 # amino length
CD = 128                 # comp_dim
PD = 128                 # prot_dim
GD = 64                  # gat_dim
H = 4                    # heads
LAT = 128                # latent
NA = 100                 # num_atom
NAM = 30                 # num_amino
LC = 3                   # conv layers
KW = 11                  # conv kernel width
ALPHA = 0.2
MASKNEG = -1.0e30
NT = N // 128            # 4 j-chunks
PADL = KW // 2


def build_core_program(debug=False):
    """Build the per-core SPMD program (identical across cores).

    debug=True builds the CoreSim-compatible variant (no Prelu — the sim
    lacks it; uses the DVE max(ax, x) leaky instead).
    """
    if debug:
        nc = bacc.Bacc(None, target_bir_lowering=False, debug=True)
    else:
        nc = bacc.Bacc(None)
    use_prelu = not debug

    # ---- DRAM I/O ----
    d_atoms = nc.dram_tensor("atoms_f", [G, N], F32, kind="ExternalInput")
    d_amask = nc.dram_tensor("atoms_mask", [G, N], F32, kind="ExternalInput")
    # ladjT_r[g, p, t, i] = additive mask for edge j->?  (j = t*128+p)
    d_ladjT = nc.dram_tensor("ladjT_r", [G, 128, NT, N], F32,
                             kind="ExternalInput")
    d_amino = nc.dram_tensor("amino_f", [G, L], F32, kind="ExternalInput")
    d_pmask = nc.dram_tensor("amino_mask", [G, L], F32, kind="ExternalInput")
    d_Eat = nc.dram_tensor("E_atom_pad", [128, CD], F32, kind="ExternalInput")
    d_Eam = nc.dram_tensor("E_amino", [NAM, PD], F32, kind="ExternalInput")
    # W_gat_r[p, h, q] = W_gat[h, p, q]
    d_Wg = nc.dram_tensor("W_gat_r", [CD, H, GD], F32, kind="ExternalInput")
    # Wa12[p, h, s]: s=0 -> (W_gat[h] @ a1_h)[p], s=1 -> (W_gat[h] @ a2_h)[p]
    d_Wa12 = nc.dram_tensor("Wa12", [CD, H, 2], F32, kind="ExternalInput")
    # W_go_r[p, c, q] = W_go[c*128+p, q]
    d_Wgo = nc.dram_tensor("W_go_r", [128, 2, CD], F32, kind="ExternalInput")
    # Wgoa[p, c, s] = (W_go @ a{s}_go)[c*128+p]
    d_Wgoa = nc.dram_tensor("Wgoa", [128, 2, 2], F32, kind="ExternalInput")
    d_Wc = nc.dram_tensor("W_comp_wT", [CD, LAT], F32, kind="ExternalInput")
    d_bc = nc.dram_tensor("W_comp_b", [LAT, 1], F32, kind="ExternalInput")
    # MiT_r[p, l, i, q] = band matrix MiT[l, i, p, q]
    d_MiT = nc.dram_tensor("MiT_r", [PD, LC, KW, PD], F32,
                           kind="ExternalInput")
    d_cb = nc.dram_tensor("conv_b", [LC, 1], F32, kind="ExternalInput")
    d_Wa = nc.dram_tensor("W_att_wT", [LAT, LAT], F32, kind="ExternalInput")
    d_ba = nc.dram_tensor("W_att_b", [LAT, 1], F32, kind="ExternalInput")
    d_pw = nc.dram_tensor("pw_cols", [LAT, 2], F32, kind="ExternalInput")
    d_pb = nc.dram_tensor("pred_b", [1, 1], F32, kind="ExternalInput")
    d_out = nc.dram_tensor("out", [G, 1], F32, kind="ExternalOutput")

    with tile.TileContext(nc) as tc:
        with (
            tc.tile_pool(name="const", bufs=1) as cpool,
            tc.tile_pool(name="work", bufs=1) as wpool,
            tc.tile_pool(name="big", bufs=2) as bpool,
            tc.tile_pool(name="adj", bufs=2) as apool,
            tc.tile_pool(name="rows", bufs=1) as rpool,
            tc.tile_pool(name="ps_sq", bufs=3, space="PSUM") as pssq,
            tc.tile_pool(name="ps_row", bufs=1, space="PSUM") as psrow,
            tc.tile_pool(name="ps_cv", bufs=3, space="PSUM") as pscv,
        ):
            # ---- constants / weights resident in SBUF ----
            ioi = cpool.tile([128, L], I32)
            nc.gpsimd.iota(ioi, pattern=[[0, L]], base=0, channel_multiplier=1)
            iof = cpool.tile([128, L], F32)
            nc.vector.tensor_copy(iof, ioi)
            ones_col = cpool.tile([128, 1], F32)
            nc.vector.memset(ones_col, 1.0)

            Eat = cpool.tile([128, CD], F32)
            nc.sync.dma_start(out=Eat, in_=d_Eat[:, :])
            Eam = cpool.tile([NAM, PD], F32)
            nc.sync.dma_start(out=Eam, in_=d_Eam[:, :])
            Wg = cpool.tile([CD, H, GD], F32)
            nc.sync.dma_start(out=Wg, in_=d_Wg[:, :, :])
            Wa12 = cpool.tile([CD, H, 2], F32)
            nc.sync.dma_start(out=Wa12, in_=d_Wa12[:, :, :])
            Wgo = cpool.tile([128, 2, CD], F32)
            nc.sync.dma_start(out=Wgo, in_=d_Wgo[:, :, :])
            Wgoa = cpool.tile([128, 2, 2], F32)
            nc.sync.dma_start(out=Wgoa, in_=d_Wgoa[:, :, :])
            Wc = cpool.tile([CD, LAT], F32)
            nc.sync.dma_start(out=Wc, in_=d_Wc[:, :])
            bc = cpool.tile([LAT, 1], F32)
            nc.sync.dma_start(out=bc, in_=d_bc[:, :])
            MiT = cpool.tile([PD, LC, KW, PD], F32)
            nc.sync.dma_start(out=MiT, in_=d_MiT[:, :, :, :])
            cb = cpool.tile([128, LC], F32)
            nc.sync.dma_start(
                out=cb,
                in_=bass.AP(tensor=d_cb, offset=0, ap=[[0, 128], [1, LC], [0, 1]]),
            )
            Wa = cpool.tile([LAT, LAT], F32)
            nc.sync.dma_start(out=Wa, in_=d_Wa[:, :])
            ba = cpool.tile([LAT, 1], F32)
            nc.sync.dma_start(out=ba, in_=d_ba[:, :])
            pw = cpool.tile([LAT, 2], F32)
            nc.sync.dma_start(out=pw, in_=d_pw[:, :])
            pb = cpool.tile([1, 1], F32)
            nc.sync.dma_start(out=pb, in_=d_pb[:, :])

            def leaky(out, in_, alpha, bias=None):
                """out = leaky_relu(in_ + bias, alpha). in_ may be PSUM."""
                if use_prelu:
                    if bias is None:
                        nc.scalar.activation(out=out, in_=in_, func=AF.Prelu,
                                             alpha=alpha)
                    else:
                        nc.scalar.activation(out=out, in_=in_, func=AF.Prelu,
                                             bias=bias, alpha=alpha)
                    return
                src = in_
                if bias is not None:
                    t = wpool.tile(list(out.shape), F32, tag="t2k",
                                   bufs=6, name="lkb")
                    nc.scalar.activation(out=t, in_=in_, func=AF.Identity,
                                         bias=bias)
                    src = t
                nc.vector.scalar_tensor_tensor(
                    out=out, in0=src, scalar=alpha, in1=src,
                    op0=AT.mult, op1=AT.max)

            def elu_into(out_ap, hp_src, rb, m, tag_sfx):
                """out = elu(hp_src * rb); hp_src PSUM (m, N), rb SBUF (m, N)."""
                hpn = wpool.tile([m, N], F32, tag="t2k", bufs=6,
                                 name="hpn" + tag_sfx)
                nc.vector.scalar_tensor_tensor(
                    out=hpn, in0=hp_src, scalar=1.0, in1=rb,
                    op0=AT.mult, op1=AT.mult)
                xm = wpool.tile([m, N], F32, tag="t2k", bufs=6,
                                name="xm" + tag_sfx)
                nc.vector.tensor_scalar(out=xm, in0=hpn, scalar1=0.0,
                                        scalar2=None, op0=AT.min)
                em = wpool.tile([m, N], F32, tag="t2k", bufs=6,
                                name="em" + tag_sfx)
                nc.scalar.activation(out=em, in_=xm, func=AF.Exp)
                nc.vector.scalar_tensor_tensor(
                    out=out_ap, in0=em, scalar=-1.0, in1=hpn,
                    op0=AT.add, op1=AT.max)

            # per-graph state carried into the fused conv / tail phases
            st = [dict() for _ in range(G)]

            for g in range(G):
                # ---------- atom embeddings via one-hot matmul ----------
                arow = rpool.tile([1, N], F32, tag="r2k", bufs=4, name="arow")
                nc.sync.dma_start(out=arow, in_=d_atoms[g : g + 1, :])
                ab = wpool.tile([128, N], F32, tag="t2k", bufs=6, name="ab")
                nc.gpsimd.partition_broadcast(ab, arow)
                ohA = wpool.tile([128, N], F32, tag="t2k", bufs=6, name="ohA")
                nc.vector.tensor_tensor(out=ohA, in0=ab, in1=iof[:, :N],
                                        op=AT.is_equal)
                avT_ps = pssq.tile([128, N], F32, tag="mm_sq", name="avT_ps")
                nc.tensor.matmul(avT_ps, Eat, ohA, start=True, stop=True)
                avT = wpool.tile([128, N], F32, tag="avT", bufs=2, name="avT")
                nc.scalar.copy(avT, avT_ps)

                ladjT = apool.tile([128, NT, N], F32, tag="ladjT", name="ladjT")
                nc.sync.dma_start(out=ladjT, in_=d_ladjT[g])

                m01 = wpool.tile([128, N], F32, tag="multi01", bufs=2, name="m01")
                m23 = wpool.tile([128, N], F32, tag="multi23", bufs=2, name="m23")
                multi = [m01, m23]

                def gat_attention(whsb, nk, src_ps, dc_ps, rowsum_sep,
                                  tag_pfx):
                    """z -> leaky -> exp -> hp (+rowsum) -> 1/rowsum bcast.

                    src_ps: PSUM (1, N) row; dc_ps: PSUM (128, NT) dst columns.
                    Returns (hp_ps, rb).
                    """
                    sd = rpool.tile([1, N], F32, tag="sd", bufs=2, name="sd")
                    nc.vector.tensor_copy(sd, src_ps)
                    srcb = wpool.tile([128, N], F32, tag="t2k", bufs=6,
                                      name="srcb" + tag_pfx)
                    nc.gpsimd.partition_broadcast(srcb, sd)
                    dcol = rpool.tile([128, NT], F32, tag="dcol", bufs=2,
                                      name="dcol")
                    nc.vector.tensor_copy(dcol, dc_ps)

                    # z = src_bcast + dst + ladj  (one fused DVE op per chunk)
                    zm = bpool.tile([128, NT, N], F32, tag="zm", name="zm")
                    for t in range(NT):
                        nc.vector.scalar_tensor_tensor(
                            out=zm[:, t, :], in0=srcb,
                            scalar=dcol[:, t : t + 1],
                            in1=ladjT[:, t, :], op0=AT.add, op1=AT.add)
                    ee = bpool.tile([128, NT, N], F32, tag="ee", name="ee")
                    leaky(ee, zm, ALPHA)
                    U = bpool.tile([128, NT, N], F32, tag="U", name="U")
                    nc.scalar.activation(out=U, in_=ee, func=AF.Exp)
                    hp_m = nk + (0 if rowsum_sep else 1)
                    hp_ps = pssq.tile([128, N], F32, tag="mm_sq", name="hp_ps")
                    for t in range(NT):
                        nc.tensor.matmul(hp_ps[:hp_m, :],
                                         whsb[:, t, :], U[:, t, :],
                                         start=(t == 0), stop=(t == NT - 1))
                    if rowsum_sep:
                        rs_ps = psrow.tile([1, N], F32, tag="ps_row",
                                           name="rs_ps")
                        for t in range(NT):
                            nc.tensor.matmul(rs_ps, ones_col, U[:, t, :],
                                             start=(t == 0), stop=(t == NT - 1))
                        zrow = rs_ps
                    else:
                        zrow = hp_ps[nk : nk + 1, :]
                    zrw = rpool.tile([1, N], F32, tag="r2k", bufs=4, name="zrw")
                    nc.vector.tensor_copy(zrw, zrow)
                    rinv = rpool.tile([1, N], F32, tag="r2k", bufs=4, name="rinv")
                    scr = rpool.tile([1, N], F32, tag="r2k", bufs=4, name="rscr")
                    nc.vector.reciprocal_approx_accurate(out=rinv, in_=zrw,
                                                         scratch=scr)
                    rb = wpool.tile([nk, N], F32, tag="t2k", bufs=6,
                                    name="rb" + tag_pfx)
                    nc.gpsimd.partition_broadcast(rb, rinv)
                    return hp_ps, rb

                # ---------- GAT heads ----------
                for h in range(H):
                    wh_ps = pssq.tile([128, NT, GD], F32, tag="mm_sq",
                                      name="wh_ps")
                    for t in range(NT):
                        nc.tensor.matmul(
                            wh_ps[:, t, :], avT[:, t * 128 : (t + 1) * 128],
                            Wg[:, h, :], start=True, stop=True)
                    whsb = wpool.tile([128, NT, GD + 1], F32, tag="t2k",
                                      bufs=6, name="whsb")
                    nc.vector.memset(whsb, 1.0)
                    nc.scalar.copy(whsb[:, :, :GD], wh_ps)

                    src_ps = psrow.tile([1, N], F32, tag="ps_row",
                                        name="src_ps")
                    nc.tensor.matmul(src_ps, Wa12[:, h, 0:1], avT,
                                     start=True, stop=True)
                    dc_ps = psrow.tile([128, NT], F32, tag="ps_dc",
                                       name="dc_ps")
                    for t in range(NT):
                        nc.tensor.matmul(dc_ps[:, t : t + 1],
                                         avT[:, t * 128 : (t + 1) * 128],
                                         Wa12[:, h, 1:2],
                                         start=True, stop=True)

                    hp_ps, rb = gat_attention(whsb, GD, src_ps, dc_ps,
                                              False, "h")
                    dsti = multi[h // 2]
                    off = (h % 2) * GD
                    elu_into(dsti[off : off + GD, :], hp_ps[:GD, :], rb,
                             GD, "h")

                # ---------- GAT output layer ----------
                wh2_ps = pssq.tile([128, NT, CD], F32, tag="mm_sq",
                                   name="wh2_ps")
                for t in range(NT):
                    for c in range(2):
                        nc.tensor.matmul(
                            wh2_ps[:, t, :],
                            multi[c][:, t * 128 : (t + 1) * 128],
                            Wgo[:, c, :], start=(c == 0), stop=(c == 1))
                wh2sb = wpool.tile([128, NT, CD], F32, tag="t2k", bufs=6,
                                   name="wh2sb")
                nc.scalar.copy(wh2sb, wh2_ps)

                src2_ps = psrow.tile([1, N], F32, tag="ps_row",
                                     name="src2_ps")
                for c in range(2):
                    nc.tensor.matmul(src2_ps, Wgoa[:, c, 0:1], multi[c],
                                     start=(c == 0), stop=(c == 1))
                dc2_ps = psrow.tile([128, NT], F32, tag="ps_dc",
                                    name="dc2_ps")
                for t in range(NT):
                    for c in range(2):
                        nc.tensor.matmul(dc2_ps[:, t : t + 1],
                                         multi[c][:, t * 128 : (t + 1) * 128],
                                         Wgoa[:, c, 1:2],
                                         start=(c == 0), stop=(c == 1))

                hp2_ps, rb2 = gat_attention(wh2sb, CD, src2_ps, dc2_ps,
                                            True, "o")
                xT = wpool.tile([CD, N], F32, tag="xT", bufs=2, name="xT")
                elu_into(xT, hp2_ps, rb2, CD, "o")

                # ---------- atoms_vec -> a_v -> comp pooling ----------
                av_ps = pssq.tile([LAT, N], F32, tag="mm_sq", name="av_ps")
                nc.tensor.matmul(av_ps, Wc, xT, start=True, stop=True)
                avec = wpool.tile([LAT, N], F32, tag="t2k", bufs=6, name="avec")
                leaky(avec, av_ps, ALPHA, bias=bc)
                av2_ps = pssq.tile([LAT, N], F32, tag="mm_sq", name="av2_ps")
                nc.tensor.matmul(av2_ps, Wa, avec, start=True, stop=True)
                a_v = wpool.tile([LAT, N], F32, tag="t2k", bufs=6, name="a_v")
                leaky(a_v, av2_ps, ALPHA, bias=ba)

                amrow = rpool.tile([1, N], F32, tag="r2k", bufs=4, name="amrow")
                nc.sync.dma_start(out=amrow, in_=d_amask[g : g + 1, :])
                amb = wpool.tile([128, N], F32, tag="t2k", bufs=6, name="amb")
                nc.gpsimd.partition_broadcast(amb, amrow)
                cscr = wpool.tile([LAT, N], F32, tag="t2k", bufs=6, name="cscr")
                comp_acc = rpool.tile([LAT, 1], F32, tag="c1", bufs=4,
                                      name="comp_acc")
                nc.vector.scalar_tensor_tensor(
                    out=cscr, in0=a_v, scalar=1.0, in1=amb,
                    op0=AT.mult, op1=AT.mult, accum_out=comp_acc)
                amscr = rpool.tile([1, N], F32, tag="r2k", bufs=4, name="amscr")
                amsum = rpool.tile([1, 1], F32, tag="c2", bufs=8, name="amsum")
                nc.vector.tensor_scalar(out=amscr, in0=amrow, scalar1=1.0,
                                        scalar2=0.0, op0=AT.mult, op1=AT.add,
                                        accum_out=amsum)
                amsb = rpool.tile([128, 1], F32, tag="c2", bufs=8, name="amsb")
                nc.gpsimd.partition_broadcast(amsb, amsum)
                amr = rpool.tile([128, 1], F32, tag="c2", bufs=8, name="amr")
                nc.vector.reciprocal(amr, amsb)
                cp = rpool.tile([128, 2], F32, tag="cp", bufs=6, name="cp")
                nc.vector.tensor_scalar(out=cp[:, 0:1], in0=comp_acc,
                                        scalar1=amr, scalar2=None,
                                        op0=AT.mult)
                st[g]["cp"] = cp

                # ---------- protein embedding (conv input) ----------
                prow = rpool.tile([1, L], F32, tag="r4k", bufs=2, name="prow")
                nc.sync.dma_start(out=prow, in_=d_amino[g : g + 1, :])
                pbm = wpool.tile([128, L], F32, tag="t4k", bufs=3, name="pbm")
                nc.gpsimd.partition_broadcast(pbm, prow)
                ohP = wpool.tile([NAM, L], F32, tag="t4k", bufs=3, name="ohP")
                nc.vector.tensor_tensor(out=ohP, in0=pbm[:NAM, :],
                                        in1=iof[:NAM, :], op=AT.is_equal)
                pv = bpool.tile([PD, L + 2 * PADL], F32, tag=f"pv{g}_0",
                                bufs=1, name="pv")
                nc.vector.memset(pv[:, :PADL], 0.0)
                nc.vector.memset(pv[:, PADL + L :], 0.0)
                for nn in range(2):
                    pvT_ps = pscv.tile([PD, 512], F32, tag="ps_cv",
                                       name="pvT_ps")
                    nc.tensor.matmul(pvT_ps, Eam,
                                     ohP[:, nn * 512 : (nn + 1) * 512],
                                     start=True, stop=True)
                    nc.scalar.copy(
                        pv[:, PADL + nn * 512 : PADL + (nn + 1) * 512], pvT_ps)
                st[g]["pv"] = pv

            # ---------- conv layers, both graphs interleaved ----------
            # (shared MiT weights stay loaded across 4 consecutive matmuls)
            for lyr in range(LC):
                pvo_l = []
                for g in range(G):
                    pvo = bpool.tile([PD, L + 2 * PADL], F32,
                                     tag=f"pv{g}_{1 - lyr % 2}", bufs=1,
                                     name="pvo")
                    nc.vector.memset(pvo[:, :PADL], 0.0)
                    nc.vector.memset(pvo[:, PADL + L :], 0.0)
                    pvo_l.append(pvo)
                for nn in range(2):
                    cv_ps = {}
                    for g in range(G):
                        cv_ps[g] = pscv.tile([PD, 512], F32, tag="ps_cv",
                                             name=f"cv_ps{g}")
                    for i in range(KW):
                        for g in range(G):
                            pv = st[g]["pv"]
                            nc.tensor.matmul(
                                cv_ps[g], MiT[:, lyr, i, :],
                                pv[:, nn * 512 + i : nn * 512 + i + 512],
                                start=(i == 0), stop=(i == KW - 1))
                    for g in range(G):
                        nc.scalar.activation(
                            out=pvo_l[g][:, PADL + nn * 512 :
                                         PADL + (nn + 1) * 512],
                            in_=cv_ps[g], func=AF.Relu,
                            bias=cb[:, lyr : lyr + 1])
                for g in range(G):
                    st[g]["pv"] = pvo_l[g]

            # ---------- p_v + prot pooling + head, per graph ----------
            for g in range(G):
                amv = st[g]["pv"][:, PADL : PADL + L]
                cp = st[g]["cp"]
                p_v = wpool.tile([LAT, L], F32, tag="t4k", bufs=3, name="p_v")
                for nn in range(2):
                    pv_ps = pscv.tile([LAT, 512], F32, tag="ps_cv",
                                      name="pv_ps")
                    nc.tensor.matmul(pv_ps, Wa,
                                     amv[:, nn * 512 : (nn + 1) * 512],
                                     start=True, stop=True)
                    leaky(p_v[:, nn * 512 : (nn + 1) * 512], pv_ps, ALPHA,
                          bias=ba)
                pmrow = rpool.tile([1, L], F32, tag="r4k", bufs=2, name="pmrow")
                nc.sync.dma_start(out=pmrow, in_=d_pmask[g : g + 1, :])
                pmb = wpool.tile([128, L], F32, tag="t4k", bufs=3, name="pmb")
                nc.gpsimd.partition_broadcast(pmb, pmrow)
                pscr = wpool.tile([LAT, L], F32, tag="t4k", bufs=3, name="pscr")
                prot_acc = rpool.tile([LAT, 1], F32, tag="c1", bufs=4,
                                      name="prot_acc")
                nc.vector.scalar_tensor_tensor(
                    out=pscr, in0=p_v, scalar=1.0, in1=pmb,
                    op0=AT.mult, op1=AT.mult, accum_out=prot_acc)
                pmscr = rpool.tile([1, L], F32, tag="r4k", bufs=2, name="pmscr")
                pmsum = rpool.tile([1, 1], F32, tag="c2", bufs=8, name="pmsum")
                nc.vector.tensor_scalar(out=pmscr, in0=pmrow, scalar1=1.0,
                                        scalar2=0.0, op0=AT.mult, op1=AT.add,
                                        accum_out=pmsum)
                pmsb = rpool.tile([128, 1], F32, tag="c2", bufs=8, name="pmsb")
                nc.gpsimd.partition_broadcast(pmsb, pmsum)
                pmr = rpool.tile([128, 1], F32, tag="c2", bufs=8, name="pmr")
                nc.vector.reciprocal(pmr, pmsb)
                nc.vector.tensor_scalar(out=cp[:, 1:2], in0=prot_acc,
                                        scalar1=pmr, scalar2=None,
                                        op0=AT.mult)

                lr2 = rpool.tile([128, 2], F32, tag="cp", bufs=6, name="lr2")
                leaky(lr2, cp, ALPHA * ALPHA)
                dscr = rpool.tile([128, 2], F32, tag="cp", bufs=6, name="dscr")
                dacc = rpool.tile([128, 1], F32, tag="c1", bufs=4, name="dacc")
                nc.vector.scalar_tensor_tensor(
                    out=dscr, in0=lr2, scalar=1.0, in1=pw,
                    op0=AT.mult, op1=AT.mult, accum_out=dacc)
                fin_ps = psrow.tile([1, 1], F32, tag="ps_row", name="fin_ps")
                nc.tensor.matmul(fin_ps, dacc, ones_col, start=True, stop=True)
                res = rpool.tile([1, 1], F32, tag="c2", bufs=8, name="res")
                nc.scalar.activation(out=res, in_=fin_ps, func=AF.Identity,
                                     bias=pb)
                nc.sync.dma_start(out=d_out[g : g + 1, :], in_=res)

    return nc


def preprocess(inputs):
    """Host-side prep: shard over cores, transpose/reshape weights."""
    atoms = np.asarray(inputs["atoms"]).astype(np.float32)
    atoms_mask = np.asarray(inputs["atoms_mask"]).astype(np.float32)
    adjacency = np.asarray(inputs["adjacency"])
    amino = np.asarray(inputs["amino"]).astype(np.float32)
    amino_mask = np.asarray(inputs["amino_mask"]).astype(np.float32)
    E_atom = np.asarray(inputs["E_atom"]).astype(np.float32)
    E_amino = np.asarray(inputs["E_amino"]).astype(np.float32)
    W_gat = np.asarray(inputs["W_gat"]).astype(np.float32)
    a_gat = np.asarray(inputs["a_gat"]).astype(np.float32)
    W_go = np.asarray(inputs["W_go"]).astype(np.float32)
    a_go = np.asarray(inputs["a_go"]).astype(np.float32)
    W_comp_w = np.asarray(inputs["W_comp_w"]).astype(np.float32)
    W_comp_b = np.asarray(inputs["W_comp_b"]).astype(np.float32)
    conv_w = np.asarray(inputs["conv_w"]).astype(np.float32)
    conv_b = np.asarray(inputs["conv_b"]).astype(np.float32)
    W_att_w = np.asarray(inputs["W_att_w"]).astype(np.float32)
    W_att_b = np.asarray(inputs["W_att_b"]).astype(np.float32)
    pred_w = np.asarray(inputs["pred_w"]).astype(np.float32)
    pred_b = np.asarray(inputs["pred_b"]).astype(np.float32)

    # additive mask, transposed, pre-tiled: [g, p, t, i] = mask(j=t*128+p, i)
    ladjT = np.where(adjacency.transpose(0, 2, 1) > 0, np.float32(0.0),
                     np.float32(MASKNEG)).astype(np.float32)
    ladjT_r = np.ascontiguousarray(
        ladjT.reshape(B, NT, 128, N).transpose(0, 2, 1, 3))

    E_atom_pad = np.zeros((128, CD), np.float32)
    E_atom_pad[:NA] = E_atom

    # conv band matrices: MiT[l, i, din, dout] = conv_w[l,0,0,i, din-dout+5]
    MiT = np.zeros((LC, KW, PD, PD), np.float32)
    din = np.arange(PD)[:, None]
    dout = np.arange(PD)[None, :]
    v = din - dout + (KW // 2)
    valid = (v >= 0) & (v < KW)
    vc = np.clip(v, 0, KW - 1)
    for lyr in range(LC):
        for i in range(KW):
            MiT[lyr, i] = np.where(valid, conv_w[lyr, 0, 0, i, vc], 0.0)
    MiT_r = np.ascontiguousarray(MiT.transpose(2, 0, 1, 3))

    W_gat_r = np.ascontiguousarray(W_gat.transpose(1, 0, 2))
    # Wa12[p, h, 0] = (W_gat[h] @ a1_h)[p]
    Wa1 = np.einsum("hpq,hq->ph", W_gat, a_gat[:, :GD])
    Wa2 = np.einsum("hpq,hq->ph", W_gat, a_gat[:, GD:])
    Wa12 = np.ascontiguousarray(np.stack([Wa1, Wa2], axis=2))
    W_go_r = np.ascontiguousarray(
        W_go.reshape(2, 128, CD).transpose(1, 0, 2))
    Wgoa = np.stack([W_go @ a_go[:CD], W_go @ a_go[CD:]], axis=1)  # (256, 2)
    Wgoa_r = np.ascontiguousarray(
        Wgoa.reshape(2, 128, 2).transpose(1, 0, 2))

    shared = {
        "E_atom_pad": E_atom_pad,
        "E_amino": np.ascontiguousarray(E_amino),
        "W_gat_r": W_gat_r,
        "Wa12": Wa12,
        "W_go_r": W_go_r,
        "Wgoa": Wgoa_r,
        "W_comp_wT": np.ascontiguousarray(W_comp_w.T),
        "W_comp_b": np.ascontiguousarray(W_comp_b[:, None]),
        "MiT_r": MiT_r,
        "conv_b": np.ascontiguousarray(conv_b.reshape(LC, 1)),
        "W_att_wT": np.ascontiguousarray(W_att_w.T),
        "W_att_b": np.ascontiguousarray(W_att_b[:, None]),
        "pw_cols": np.ascontiguousarray(
            np.stack([pred_w[0, :LAT], pred_w[0, LAT:]], axis=1)),
        "pred_b": np.ascontiguousarray(pred_b.reshape(1, 1)),
    }
    in_maps = []
    for c in range(NCORES):
        sl = slice(c * G, (c + 1) * G)
        m = dict(shared)
        m["atoms_f"] = np.ascontiguousarray(atoms[sl])
        m["atoms_mask"] = np.ascontiguousarray(atoms_mask[sl])
        m["ladjT_r"] = np.ascontiguousarray(ladjT_r[sl])
        m["amino_f"] = np.ascontiguousarray(amino[sl])
        m["amino_mask"] = np.ascontiguousarray(amino_mask[sl])
        in_maps.append(m)
    return in_maps


_CACHED_NC = None


def kernel(**inputs) -> np.ndarray:
    global _CACHED_NC
    from concourse.bass_utils import run_bass_kernel_spmd

    if _CACHED_NC is None:
        nc = build_core_program()
        nc.finalize()
        _CACHED_NC = nc
    nc = _CACHED_NC
    in_maps = preprocess(inputs)
    res = run_bass_kernel_spmd(nc, in_maps, core_ids=list(range(NCORES)))
    out = np.concatenate([res.results[c]["out"] for c in range(NCORES)], axis=0)
    return out.astype(np.float32)


# revision 16
# speedup vs baseline: 1.7769x; 1.0077x over previous
"""Trainium2 Bass kernel for BiDACPI (GAT + CNN + bidirectional attention).

Data-parallel over batch b=16 across 8 NeuronCores (2 graphs per core).
Self-contained: hardcodes all shapes; host-side preprocessing only reshapes /
transposes weights and converts index tensors.
"""
import numpy as np

import concourse.bass as bass
import concourse.mybir as mybir
import concourse.tile as tile
from concourse import bacc

F32 = mybir.dt.float32
I32 = mybir.dt.int32
AT = mybir.AluOpType
AF = mybir.ActivationFunctionType

# Problem constants
B = 16
NCORES = 8
G = B // NCORES          # graphs per core
N = 512                  # atoms per graph
L = 1024                 # amino length
CD = 128                 # comp_dim
PD = 128                 # prot_dim
GD = 64                  # gat_dim
H = 4                    # heads
LAT = 128                # latent
NA = 100                 # num_atom
NAM = 30                 # num_amino
LC = 3                   # conv layers
KW = 11                  # conv kernel width
ALPHA = 0.2
MASKNEG = -1.0e30
NT = N // 128            # 4 j-chunks
PADL = KW // 2


def build_core_program(debug=False):
    """Build the per-core SPMD program (identical across cores).

    debug=True builds the CoreSim-compatible variant (no Prelu — the sim
    lacks it; uses the DVE max(ax, x) leaky instead).
    """
    if debug:
        nc = bacc.Bacc(None, target_bir_lowering=False, debug=True)
    else:
        nc = bacc.Bacc(None)
    use_prelu = not debug

    # ---- DRAM I/O ----
    d_atoms = nc.dram_tensor("atoms_f", [G, N], F32, kind="ExternalInput")
    d_amask = nc.dram_tensor("atoms_mask", [G, N], F32, kind="ExternalInput")
    # ladjT_r[g, p, t, i] = additive mask for edge j->?  (j = t*128+p)
    d_ladjT = nc.dram_tensor("ladjT_r", [G, 128, NT, N], F32,
                             kind="ExternalInput")
    d_amino = nc.dram_tensor("amino_f", [G, L], F32, kind="ExternalInput")
    d_pmask = nc.dram_tensor("amino_mask", [G, L], F32, kind="ExternalInput")
    d_Eat = nc.dram_tensor("E_atom_pad", [128, CD], F32, kind="ExternalInput")
    d_Eam = nc.dram_tensor("E_amino", [NAM, PD], F32, kind="ExternalInput")
    # W_gat_r[p, h, q] = W_gat[h, p, q]
    d_Wg = nc.dram_tensor("W_gat_r", [CD, H, GD], F32, kind="ExternalInput")
    # Wa12[p, h, s]: s=0 -> (W_gat[h] @ a1_h)[p], s=1 -> (W_gat[h] @ a2_h)[p]
    d_Wa12 = nc.dram_tensor("Wa12", [CD, H, 2], F32, kind="ExternalInput")
    # a2_rows[0, h, q] = a_gat[h, GD+q]; a2go_row[0, q] = a_go[CD+q]
    d_a2r = nc.dram_tensor("a2_rows", [1, H, GD], F32, kind="ExternalInput")
    d_a2go = nc.dram_tensor("a2go_row", [1, CD], F32, kind="ExternalInput")
    # W_go_r[p, c, q] = W_go[c*128+p, q]
    d_Wgo = nc.dram_tensor("W_go_r", [128, 2, CD], F32, kind="ExternalInput")
    # Wgoa[p, c, s] = (W_go @ a{s}_go)[c*128+p]
    d_Wgoa = nc.dram_tensor("Wgoa", [128, 2, 2], F32, kind="ExternalInput")
    d_Wc = nc.dram_tensor("W_comp_wT", [CD, LAT], F32, kind="ExternalInput")
    d_bc = nc.dram_tensor("W_comp_b", [LAT, 1], F32, kind="ExternalInput")
    # MiT_r[p, l, i, q] = band matrix MiT[l, i, p, q]
    d_MiT = nc.dram_tensor("MiT_r", [PD, LC, KW, PD], F32,
                           kind="ExternalInput")
    d_cb = nc.dram_tensor("conv_b", [LC, 1], F32, kind="ExternalInput")
    d_Wa = nc.dram_tensor("W_att_wT", [LAT, LAT], F32, kind="ExternalInput")
    d_ba = nc.dram_tensor("W_att_b", [LAT, 1], F32, kind="ExternalInput")
    d_pw = nc.dram_tensor("pw_cols", [LAT, 2], F32, kind="ExternalInput")
    d_pb = nc.dram_tensor("pred_b", [1, 1], F32, kind="ExternalInput")
    d_out = nc.dram_tensor("out", [G, 1], F32, kind="ExternalOutput")

    with tile.TileContext(nc) as tc:
        with (
            tc.tile_pool(name="const", bufs=1) as cpool,
            tc.tile_pool(name="work", bufs=1) as wpool,
            tc.tile_pool(name="big", bufs=2) as bpool,
            tc.tile_pool(name="adj", bufs=2) as apool,
            tc.tile_pool(name="rows", bufs=1) as rpool,
            tc.tile_pool(name="ps_sq", bufs=2, space="PSUM") as pssq,
            tc.tile_pool(name="ps_row", bufs=1, space="PSUM") as psrow,
            tc.tile_pool(name="ps_cv", bufs=4, space="PSUM") as pscv,
            tc.tile_pool(name="ps_wh", bufs=1, space="PSUM") as pswh,
        ):
            # ---- constants / weights resident in SBUF ----
            ioi = cpool.tile([128, L], I32)
            nc.gpsimd.iota(ioi, pattern=[[0, L]], base=0, channel_multiplier=1)
            iof = cpool.tile([128, L], F32)
            nc.vector.tensor_copy(iof, ioi)
            ones_col = cpool.tile([128, 1], F32)
            nc.vector.memset(ones_col, 1.0)

            Eat = cpool.tile([128, CD], F32)
            nc.sync.dma_start(out=Eat, in_=d_Eat[:, :])
            Eam = cpool.tile([NAM, PD], F32)
            nc.sync.dma_start(out=Eam, in_=d_Eam[:, :])
            Wg = cpool.tile([CD, H, GD], F32)
            nc.sync.dma_start(out=Wg, in_=d_Wg[:, :, :])
            Wa12 = cpool.tile([CD, H, 2], F32)
            nc.sync.dma_start(out=Wa12, in_=d_Wa12[:, :, :])
            a2r = cpool.tile([1, H, GD], F32)
            nc.sync.dma_start(out=a2r, in_=d_a2r[:, :, :])
            a2go = cpool.tile([1, CD], F32)
            nc.sync.dma_start(out=a2go, in_=d_a2go[:, :])
            Wgo = cpool.tile([128, 2, CD], F32)
            nc.sync.dma_start(out=Wgo, in_=d_Wgo[:, :, :])
            Wgoa = cpool.tile([128, 2, 2], F32)
            nc.sync.dma_start(out=Wgoa, in_=d_Wgoa[:, :, :])
            Wc = cpool.tile([CD, LAT], F32)
            nc.sync.dma_start(out=Wc, in_=d_Wc[:, :])
            bc = cpool.tile([LAT, 1], F32)
            nc.sync.dma_start(out=bc, in_=d_bc[:, :])
            MiT = cpool.tile([PD, LC, KW, PD], F32)
            nc.scalar.dma_start(out=MiT, in_=d_MiT[:, :, :, :])
            cb = cpool.tile([128, LC], F32)
            nc.sync.dma_start(
                out=cb,
                in_=bass.AP(tensor=d_cb, offset=0, ap=[[0, 128], [1, LC], [0, 1]]),
            )
            Wa = cpool.tile([LAT, LAT], F32)
            nc.scalar.dma_start(out=Wa, in_=d_Wa[:, :])
            ba = cpool.tile([LAT, 1], F32)
            nc.sync.dma_start(out=ba, in_=d_ba[:, :])
            pw = cpool.tile([LAT, 2], F32)
            nc.sync.dma_start(out=pw, in_=d_pw[:, :])
            pb = cpool.tile([1, 1], F32)
            nc.sync.dma_start(out=pb, in_=d_pb[:, :])

            def leaky(out, in_, alpha, bias=None):
                """out = leaky_relu(in_ + bias, alpha). in_ may be PSUM."""
                if use_prelu:
                    if bias is None:
                        nc.scalar.activation(out=out, in_=in_, func=AF.Prelu,
                                             alpha=alpha)
                    else:
                        nc.scalar.activation(out=out, in_=in_, func=AF.Prelu,
                                             bias=bias, alpha=alpha)
                    return
                src = in_
                if bias is not None:
                    t = wpool.tile(list(out.shape), F32, tag="t2k",
                                   bufs=6, name="lkb")
                    nc.scalar.activation(out=t, in_=in_, func=AF.Identity,
                                         bias=bias)
                    src = t
                nc.vector.scalar_tensor_tensor(
                    out=out, in0=src, scalar=alpha, in1=src,
                    op0=AT.mult, op1=AT.max)

            def elu_into(out_ap, hp_src, rb, m, tag_sfx):
                """out = elu(hp_src * rb); hp_src PSUM (m, N), rb SBUF (m, N)."""
                hpn = wpool.tile([m, N], F32, tag="t2k", bufs=6,
                                 name="hpn" + tag_sfx)
                nc.vector.scalar_tensor_tensor(
                    out=hpn, in0=hp_src, scalar=1.0, in1=rb,
                    op0=AT.mult, op1=AT.mult)
                xm = wpool.tile([m, N], F32, tag="t2k", bufs=6,
                                name="xm" + tag_sfx)
                nc.vector.tensor_scalar(out=xm, in0=hpn, scalar1=0.0,
                                        scalar2=None, op0=AT.min)
                em = wpool.tile([m, N], F32, tag="t2k", bufs=6,
                                name="em" + tag_sfx)
                nc.scalar.activation(out=em, in_=xm, func=AF.Exp)
                nc.vector.scalar_tensor_tensor(
                    out=out_ap, in0=em, scalar=-1.0, in1=hpn,
                    op0=AT.add, op1=AT.max)

            # per-graph state carried into the fused conv / tail phases
            st = [dict() for _ in range(G)]

            for g in range(G):
                # ---------- atom embeddings via one-hot matmul ----------
                arow = rpool.tile([1, N], F32, tag="r2k", bufs=4, name="arow")
                nc.sync.dma_start(out=arow, in_=d_atoms[g : g + 1, :])
                ab = wpool.tile([128, N], F32, tag="t2k", bufs=6, name="ab")
                nc.gpsimd.partition_broadcast(ab, arow)
                ohA = wpool.tile([128, N], F32, tag="t2k", bufs=6, name="ohA")
                nc.vector.tensor_tensor(out=ohA, in0=ab, in1=iof[:, :N],
                                        op=AT.is_equal)
                avT_ps = pssq.tile([128, N], F32, tag="mm_sq", name="avT_ps")
                nc.tensor.matmul(avT_ps, Eat, ohA, start=True, stop=True)
                avT = wpool.tile([128, N], F32, tag="avT", bufs=2, name="avT")
                nc.scalar.copy(avT, avT_ps)

                ladjT = apool.tile([128, NT, N], F32, tag="ladjT", name="ladjT")
                nc.gpsimd.dma_start(out=ladjT, in_=d_ladjT[g])

                m01 = wpool.tile([128, N], F32, tag="multi01", bufs=2, name="m01")
                m23 = wpool.tile([128, N], F32, tag="multi23", bufs=2, name="m23")
                multi = [m01, m23]

                def gat_attention(whsb, nk, src_ps, dcol, rowsum_sep,
                                  tag_pfx):
                    """z -> leaky -> exp -> hp (+rowsum) -> 1/rowsum bcast.

                    src_ps: PSUM (1, N) row; dcol: SBUF (128, NT) dst columns.
                    Returns (hp_ps, rb).
                    """
                    sd = rpool.tile([1, N], F32, tag="sd", bufs=2, name="sd")
                    nc.vector.tensor_copy(sd, src_ps)
                    srcb = wpool.tile([128, N], F32, tag="t2k", bufs=6,
                                      name="srcb" + tag_pfx)
                    nc.gpsimd.partition_broadcast(srcb, sd)

                    # z = src_bcast + dst + ladj  (one fused DVE op per chunk)
                    zm = bpool.tile([128, NT, N], F32, tag="zm", name="zm")
                    for t in range(NT):
                        nc.vector.scalar_tensor_tensor(
                            out=zm[:, t, :], in0=srcb,
                            scalar=dcol[:, t : t + 1],
                            in1=ladjT[:, t, :], op0=AT.add, op1=AT.add)
                    ee = bpool.tile([128, NT, N], F32, tag="ee", name="ee")
                    leaky(ee, zm, ALPHA)
                    U = bpool.tile([128, NT, N], F32, tag="U", name="U")
                    nc.scalar.activation(out=U, in_=ee, func=AF.Exp)
                    hp_m = nk + (0 if rowsum_sep else 1)
                    hp_ps = pssq.tile([128, N], F32, tag="mm_sq", name="hp_ps")
                    for t in range(NT):
                        nc.tensor.matmul(hp_ps[:hp_m, :],
                                         whsb[:, t, :], U[:, t, :],
                                         start=(t == 0), stop=(t == NT - 1))
                    if rowsum_sep:
                        rs_ps = psrow.tile([1, N], F32, tag="ps_row",
                                           name="rs_ps")
                        for t in range(NT):
                            nc.tensor.matmul(rs_ps, ones_col, U[:, t, :],
                                             start=(t == 0), stop=(t == NT - 1))
                        zrow = rs_ps
                    else:
                        zrow = hp_ps[nk : nk + 1, :]
                    zrw = rpool.tile([1, N], F32, tag="r2k", bufs=4, name="zrw")
                    nc.vector.tensor_copy(zrw, zrow)
                    rinv = rpool.tile([1, N], F32, tag="r2k", bufs=4, name="rinv")
                    scr = rpool.tile([1, N], F32, tag="r2k", bufs=4, name="rscr")
                    nc.vector.reciprocal_approx_accurate(out=rinv, in_=zrw,
                                                         scratch=scr)
                    rb = wpool.tile([nk, N], F32, tag="t2k", bufs=6,
                                    name="rb" + tag_pfx)
                    nc.gpsimd.partition_broadcast(rb, rinv)
                    return hp_ps, rb

                # ---------- GAT heads ----------
                # all heads' Wh chunks, batched so each avT chunk (lhsT) is
                # loaded once and reused for the 4 heads
                whsb_all = wpool.tile([128, NT, H, GD + 1], F32,
                                      tag="whsb_all", bufs=1, name="whsb_all")
                nc.vector.memset(whsb_all, 1.0)
                for half in range(2):
                    wh_all = pswh.tile([128, 2, H, GD], F32, tag="wh_all",
                                       name="wh_all")
                    for t2 in range(2):
                        t = half * 2 + t2
                        for h in range(H):
                            nc.tensor.matmul(
                                wh_all[:, t2, h, :],
                                avT[:, t * 128 : (t + 1) * 128],
                                Wg[:, h, :], start=True, stop=True)
                    nc.scalar.copy(
                        whsb_all[:, half * 2 : half * 2 + 2, :, :GD], wh_all)

                for h in range(H):
                    whsb = whsb_all[:, :, h, :]
                    src_ps = psrow.tile([1, N], F32, tag="ps_row",
                                        name="src_ps")
                    nc.tensor.matmul(src_ps, Wa12[:, h, 0:1], avT,
                                     start=True, stop=True)
                    a2b = wpool.tile([128, GD], F32, tag="a2b", bufs=2,
                                     name="a2b")
                    nc.gpsimd.partition_broadcast(a2b, a2r[0:1, h, :])
                    dcol = rpool.tile([128, NT], F32, tag="dcol", bufs=2,
                                      name="dcol")
                    dsc = rpool.tile([128, GD], F32, tag="dsc", bufs=2,
                                     name="dsc")
                    for t in range(NT):
                        nc.vector.scalar_tensor_tensor(
                            out=dsc, in0=whsb[:, t, :GD], scalar=1.0,
                            in1=a2b, op0=AT.mult, op1=AT.mult,
                            accum_out=dcol[:, t : t + 1])

                    hp_ps, rb = gat_attention(whsb, GD, src_ps, dcol,
                                              False, "h")
                    dsti = multi[h // 2]
                    off = (h % 2) * GD
                    elu_into(dsti[off : off + GD, :], hp_ps[:GD, :], rb,
                             GD, "h")

                # ---------- GAT output layer ----------
                wh2_ps = pssq.tile([128, NT, CD], F32, tag="mm_sq",
                                   name="wh2_ps")
                for t in range(NT):
                    for c in range(2):
                        nc.tensor.matmul(
                            wh2_ps[:, t, :],
                            multi[c][:, t * 128 : (t + 1) * 128],
                            Wgo[:, c, :], start=(c == 0), stop=(c == 1))
                wh2sb = wpool.tile([128, NT, CD], F32, tag="t2k", bufs=6,
                                   name="wh2sb")
                nc.scalar.copy(wh2sb, wh2_ps)

                src2_ps = psrow.tile([1, N], F32, tag="ps_row",
                                     name="src2_ps")
                for c in range(2):
                    nc.tensor.matmul(src2_ps, Wgoa[:, c, 0:1], multi[c],
                                     start=(c == 0), stop=(c == 1))
                a2gob = wpool.tile([128, CD], F32, tag="a2b", bufs=2,
                                   name="a2gob")
                nc.gpsimd.partition_broadcast(a2gob, a2go)
                dcol2 = rpool.tile([128, NT], F32, tag="dcol", bufs=2,
                                   name="dcol2")
                dsc2 = rpool.tile([128, CD], F32, tag="dsc", bufs=2,
                                  name="dsc2")
                for t in range(NT):
                    nc.vector.scalar_tensor_tensor(
                        out=dsc2, in0=wh2sb[:, t, :], scalar=1.0,
                        in1=a2gob, op0=AT.mult, op1=AT.mult,
                        accum_out=dcol2[:, t : t + 1])

                hp2_ps, rb2 = gat_attention(wh2sb, CD, src2_ps, dcol2,
                                            True, "o")
                xT = wpool.tile([CD, N], F32, tag="xT", bufs=2, name="xT")
                elu_into(xT, hp2_ps, rb2, CD, "o")

                # ---------- atoms_vec -> a_v -> comp pooling ----------
                av_ps = pssq.tile([LAT, N], F32, tag="mm_sq", name="av_ps")
                nc.tensor.matmul(av_ps, Wc, xT, start=True, stop=True)
                avec = wpool.tile([LAT, N], F32, tag="t2k", bufs=6, name="avec")
                leaky(avec, av_ps, ALPHA, bias=bc)
                av2_ps = pssq.tile([LAT, N], F32, tag="mm_sq", name="av2_ps")
                nc.tensor.matmul(av2_ps, Wa, avec, start=True, stop=True)
                a_v = wpool.tile([LAT, N], F32, tag="t2k", bufs=6, name="a_v")
                leaky(a_v, av2_ps, ALPHA, bias=ba)

                amrow = rpool.tile([1, N], F32, tag="r2k", bufs=4, name="amrow")
                nc.sync.dma_start(out=amrow, in_=d_amask[g : g + 1, :])
                amb = wpool.tile([128, N], F32, tag="t2k", bufs=6, name="amb")
                nc.gpsimd.partition_broadcast(amb, amrow)
                cscr = wpool.tile([LAT, N], F32, tag="t2k", bufs=6, name="cscr")
                comp_acc = rpool.tile([LAT, 1], F32, tag="c1", bufs=4,
                                      name="comp_acc")
                nc.vector.scalar_tensor_tensor(
                    out=cscr, in0=a_v, scalar=1.0, in1=amb,
                    op0=AT.mult, op1=AT.mult, accum_out=comp_acc)
                amscr = rpool.tile([1, N], F32, tag="r2k", bufs=4, name="amscr")
                amsum = rpool.tile([1, 1], F32, tag="c2", bufs=8, name="amsum")
                nc.vector.tensor_scalar(out=amscr, in0=amrow, scalar1=1.0,
                                        scalar2=0.0, op0=AT.mult, op1=AT.add,
                                        accum_out=amsum)
                amsb = rpool.tile([128, 1], F32, tag="c2", bufs=8, name="amsb")
                nc.gpsimd.partition_broadcast(amsb, amsum)
                amr = rpool.tile([128, 1], F32, tag="c2", bufs=8, name="amr")
                nc.vector.reciprocal(amr, amsb)
                cp = rpool.tile([128, 2], F32, tag="cp", bufs=6, name="cp")
                nc.vector.tensor_scalar(out=cp[:, 0:1], in0=comp_acc,
                                        scalar1=amr, scalar2=None,
                                        op0=AT.mult)
                st[g]["cp"] = cp

                # ---------- protein embedding (conv input) ----------
                prow = rpool.tile([1, L], F32, tag="r4k", bufs=2, name="prow")
                nc.sync.dma_start(out=prow, in_=d_amino[g : g + 1, :])
                pbm = wpool.tile([128, L], F32, tag="t4k", bufs=3, name="pbm")
                nc.gpsimd.partition_broadcast(pbm, prow)
                ohP = wpool.tile([NAM, L], F32, tag="t4k", bufs=3, name="ohP")
                nc.vector.tensor_tensor(out=ohP, in0=pbm[:NAM, :],
                                        in1=iof[:NAM, :], op=AT.is_equal)
                pv = bpool.tile([PD, L + 2 * PADL], F32, tag=f"pv{g}_0",
                                bufs=1, name="pv")
                nc.vector.memset(pv[:, :PADL], 0.0)
                nc.vector.memset(pv[:, PADL + L :], 0.0)
                for nn in range(2):
                    pvT_ps = pscv.tile([PD, 512], F32, tag="ps_cv",
                                       name="pvT_ps")
                    nc.tensor.matmul(pvT_ps, Eam,
                                     ohP[:, nn * 512 : (nn + 1) * 512],
                                     start=True, stop=True)
                    nc.scalar.copy(
                        pv[:, PADL + nn * 512 : PADL + (nn + 1) * 512], pvT_ps)
                st[g]["pv"] = pv

            # ---------- conv layers, both graphs interleaved ----------
            # (shared MiT weights stay loaded across 4 consecutive matmuls)
            for lyr in range(LC):
                pvo_l = []
                for g in range(G):
                    pvo = bpool.tile([PD, L + 2 * PADL], F32,
                                     tag=f"pv{g}_{1 - lyr % 2}", bufs=1,
                                     name="pvo")
                    nc.vector.memset(pvo[:, :PADL], 0.0)
                    nc.vector.memset(pvo[:, PADL + L :], 0.0)
                    pvo_l.append(pvo)
                cv_ps = {}
                for g in range(G):
                    for nn in range(2):
                        cv_ps[(g, nn)] = pscv.tile([PD, 512], F32,
                                                   tag="ps_cv",
                                                   name=f"cv{g}{nn}")
                for i in range(KW):
                    for g in range(G):
                        pv = st[g]["pv"]
                        for nn in range(2):
                            nc.tensor.matmul(
                                cv_ps[(g, nn)], MiT[:, lyr, i, :],
                                pv[:, nn * 512 + i : nn * 512 + i + 512],
                                start=(i == 0), stop=(i == KW - 1))
                for g in range(G):
                    for nn in range(2):
                        nc.scalar.activation(
                            out=pvo_l[g][:, PADL + nn * 512 :
                                         PADL + (nn + 1) * 512],
                            in_=cv_ps[(g, nn)], func=AF.Relu,
                            bias=cb[:, lyr : lyr + 1])
                for g in range(G):
                    st[g]["pv"] = pvo_l[g]

            # ---------- p_v + prot pooling + head, per graph ----------
            for g in range(G):
                amv = st[g]["pv"][:, PADL : PADL + L]
                cp = st[g]["cp"]
                p_v = wpool.tile([LAT, L], F32, tag="t4k", bufs=3, name="p_v")
                for nn in range(2):
                    pv_ps = pscv.tile([LAT, 512], F32, tag="ps_cv",
                                      name="pv_ps")
                    nc.tensor.matmul(pv_ps, Wa,
                                     amv[:, nn * 512 : (nn + 1) * 512],
                                     start=True, stop=True)
                    leaky(p_v[:, nn * 512 : (nn + 1) * 512], pv_ps, ALPHA,
                          bias=ba)
                pmrow = rpool.tile([1, L], F32, tag="r4k", bufs=2, name="pmrow")
                nc.sync.dma_start(out=pmrow, in_=d_pmask[g : g + 1, :])
                pmb = wpool.tile([128, L], F32, tag="t4k", bufs=3, name="pmb")
                nc.gpsimd.partition_broadcast(pmb, pmrow)
                pscr = wpool.tile([LAT, L], F32, tag="t4k", bufs=3, name="pscr")
                prot_acc = rpool.tile([LAT, 1], F32, tag="c1", bufs=4,
                                      name="prot_acc")
                nc.vector.scalar_tensor_tensor(
                    out=pscr, in0=p_v, scalar=1.0, in1=pmb,
                    op0=AT.mult, op1=AT.mult, accum_out=prot_acc)
                pmscr = rpool.tile([1, L], F32, tag="r4k", bufs=2, name="pmscr")
                pmsum = rpool.tile([1, 1], F32, tag="c2", bufs=8, name="pmsum")
                nc.vector.tensor_scalar(out=pmscr, in0=pmrow, scalar1=1.0,
                                        scalar2=0.0, op0=AT.mult, op1=AT.add,
                                        accum_out=pmsum)
                pmsb = rpool.tile([128, 1], F32, tag="c2", bufs=8, name="pmsb")
                nc.gpsimd.partition_broadcast(pmsb, pmsum)
                pmr = rpool.tile([128, 1], F32, tag="c2", bufs=8, name="pmr")
                nc.vector.reciprocal(pmr, pmsb)
                nc.vector.tensor_scalar(out=cp[:, 1:2], in0=prot_acc,
                                        scalar1=pmr, scalar2=None,
                                        op0=AT.mult)

                lr2 = rpool.tile([128, 2], F32, tag="cp", bufs=6, name="lr2")
                leaky(lr2, cp, ALPHA * ALPHA)
                dscr = rpool.tile([128, 2], F32, tag="cp", bufs=6, name="dscr")
                dacc = rpool.tile([128, 1], F32, tag="c1", bufs=4, name="dacc")
                nc.vector.scalar_tensor_tensor(
                    out=dscr, in0=lr2, scalar=1.0, in1=pw,
                    op0=AT.mult, op1=AT.mult, accum_out=dacc)
                fin_ps = psrow.tile([1, 1], F32, tag="ps_row", name="fin_ps")
                nc.tensor.matmul(fin_ps, dacc, ones_col, start=True, stop=True)
                res = rpool.tile([1, 1], F32, tag="c2", bufs=8, name="res")
                nc.scalar.activation(out=res, in_=fin_ps, func=AF.Identity,
                                     bias=pb)
                nc.sync.dma_start(out=d_out[g : g + 1, :], in_=res)

    return nc


def preprocess(inputs):
    """Host-side prep: shard over cores, transpose/reshape weights."""
    atoms = np.asarray(inputs["atoms"]).astype(np.float32)
    atoms_mask = np.asarray(inputs["atoms_mask"]).astype(np.float32)
    adjacency = np.asarray(inputs["adjacency"])
    amino = np.asarray(inputs["amino"]).astype(np.float32)
    amino_mask = np.asarray(inputs["amino_mask"]).astype(np.float32)
    E_atom = np.asarray(inputs["E_atom"]).astype(np.float32)
    E_amino = np.asarray(inputs["E_amino"]).astype(np.float32)
    W_gat = np.asarray(inputs["W_gat"]).astype(np.float32)
    a_gat = np.asarray(inputs["a_gat"]).astype(np.float32)
    W_go = np.asarray(inputs["W_go"]).astype(np.float32)
    a_go = np.asarray(inputs["a_go"]).astype(np.float32)
    W_comp_w = np.asarray(inputs["W_comp_w"]).astype(np.float32)
    W_comp_b = np.asarray(inputs["W_comp_b"]).astype(np.float32)
    conv_w = np.asarray(inputs["conv_w"]).astype(np.float32)
    conv_b = np.asarray(inputs["conv_b"]).astype(np.float32)
    W_att_w = np.asarray(inputs["W_att_w"]).astype(np.float32)
    W_att_b = np.asarray(inputs["W_att_b"]).astype(np.float32)
    pred_w = np.asarray(inputs["pred_w"]).astype(np.float32)
    pred_b = np.asarray(inputs["pred_b"]).astype(np.float32)

    # additive mask, transposed, pre-tiled: [g, p, t, i] = mask(j=t*128+p, i)
    ladjT = np.where(adjacency.transpose(0, 2, 1) > 0, np.float32(0.0),
                     np.float32(MASKNEG)).astype(np.float32)
    ladjT_r = np.ascontiguousarray(
        ladjT.reshape(B, NT, 128, N).transpose(0, 2, 1, 3))

    E_atom_pad = np.zeros((128, CD), np.float32)
    E_atom_pad[:NA] = E_atom

    # conv band matrices: MiT[l, i, din, dout] = conv_w[l,0,0,i, din-dout+5]
    MiT = np.zeros((LC, KW, PD, PD), np.float32)
    din = np.arange(PD)[:, None]
    dout = np.arange(PD)[None, :]
    v = din - dout + (KW // 2)
    valid = (v >= 0) & (v < KW)
    vc = np.clip(v, 0, KW - 1)
    for lyr in range(LC):
        for i in range(KW):
            MiT[lyr, i] = np.where(valid, conv_w[lyr, 0, 0, i, vc], 0.0)
    MiT_r = np.ascontiguousarray(MiT.transpose(2, 0, 1, 3))

    W_gat_r = np.ascontiguousarray(W_gat.transpose(1, 0, 2))
    # Wa12[p, h, 0] = (W_gat[h] @ a1_h)[p]
    Wa1 = np.einsum("hpq,hq->ph", W_gat, a_gat[:, :GD])
    Wa2 = np.einsum("hpq,hq->ph", W_gat, a_gat[:, GD:])
    Wa12 = np.ascontiguousarray(np.stack([Wa1, Wa2], axis=2))
    W_go_r = np.ascontiguousarray(
        W_go.reshape(2, 128, CD).transpose(1, 0, 2))
    Wgoa = np.stack([W_go @ a_go[:CD], W_go @ a_go[CD:]], axis=1)  # (256, 2)
    Wgoa_r = np.ascontiguousarray(
        Wgoa.reshape(2, 128, 2).transpose(1, 0, 2))

    shared = {
        "E_atom_pad": E_atom_pad,
        "E_amino": np.ascontiguousarray(E_amino),
        "W_gat_r": W_gat_r,
        "Wa12": Wa12,
        "a2_rows": np.ascontiguousarray(a_gat[:, GD:][None, :, :]),
        "a2go_row": np.ascontiguousarray(a_go[CD:][None, :]),
        "W_go_r": W_go_r,
        "Wgoa": Wgoa_r,
        "W_comp_wT": np.ascontiguousarray(W_comp_w.T),
        "W_comp_b": np.ascontiguousarray(W_comp_b[:, None]),
        "MiT_r": MiT_r,
        "conv_b": np.ascontiguousarray(conv_b.reshape(LC, 1)),
        "W_att_wT": np.ascontiguousarray(W_att_w.T),
        "W_att_b": np.ascontiguousarray(W_att_b[:, None]),
        "pw_cols": np.ascontiguousarray(
            np.stack([pred_w[0, :LAT], pred_w[0, LAT:]], axis=1)),
        "pred_b": np.ascontiguousarray(pred_b.reshape(1, 1)),
    }
    in_maps = []
    for c in range(NCORES):
        sl = slice(c * G, (c + 1) * G)
        m = dict(shared)
        m["atoms_f"] = np.ascontiguousarray(atoms[sl])
        m["atoms_mask"] = np.ascontiguousarray(atoms_mask[sl])
        m["ladjT_r"] = np.ascontiguousarray(ladjT_r[sl])
        m["amino_f"] = np.ascontiguousarray(amino[sl])
        m["amino_mask"] = np.ascontiguousarray(amino_mask[sl])
        in_maps.append(m)
    return in_maps


_CACHED_NC = None


def kernel(**inputs) -> np.ndarray:
    global _CACHED_NC
    from concourse.bass_utils import run_bass_kernel_spmd

    if _CACHED_NC is None:
        nc = build_core_program()
        nc.finalize()
        _CACHED_NC = nc
    nc = _CACHED_NC
    in_maps = preprocess(inputs)
    res = run_bass_kernel_spmd(nc, in_maps, core_ids=list(range(NCORES)))
    out = np.concatenate([res.results[c]["out"] for c in range(NCORES)], axis=0)
    return out.astype(np.float32)


# revision 18
# speedup vs baseline: 2.2038x; 1.2402x over previous
"""Trainium2 Bass kernel for BiDACPI (GAT + CNN + bidirectional attention).

Data-parallel over batch b=16 across 8 NeuronCores (2 graphs per core).
Self-contained: hardcodes all shapes; host-side preprocessing only reshapes /
transposes weights and converts index tensors.
"""
import numpy as np

import concourse.bass as bass
import concourse.mybir as mybir
import concourse.tile as tile
from concourse import bacc

F32 = mybir.dt.float32
BF16 = mybir.dt.bfloat16
I32 = mybir.dt.int32
AT = mybir.AluOpType
AF = mybir.ActivationFunctionType

# Problem constants
B = 16
NCORES = 8
G = B // NCORES          # graphs per core
N = 512                  # atoms per graph
L = 1024                 # amino length
CD = 128                 # comp_dim
PD = 128                 # prot_dim
GD = 64                  # gat_dim
H = 4                    # heads
LAT = 128                # latent
NA = 100                 # num_atom
NAM = 30                 # num_amino
LC = 3                   # conv layers
KW = 11                  # conv kernel width
ALPHA = 0.2
MASKNEG = -1.0e30
NT = N // 128            # 4 j-chunks
PADL = KW // 2


def build_core_program(debug=False, mm_bf16=True):
    """Build the per-core SPMD program (identical across cores).

    debug=True builds the CoreSim-compatible variant (no Prelu — the sim
    lacks it; uses the DVE max(ax, x) leaky instead).
    mm_bf16=True runs matmul operands in bf16 (f32 PSUM accumulation) — on
    this part an f32 matmul self-loads its weights serially (~2x cost), so
    bf16 halves TensorE time.
    """
    if debug:
        nc = bacc.Bacc(None, target_bir_lowering=False, debug=True)
    else:
        nc = bacc.Bacc(None)
    use_prelu = not debug
    MD = BF16 if mm_bf16 else F32

    # ---- DRAM I/O ----
    d_atoms = nc.dram_tensor("atoms_f", [G, N], F32, kind="ExternalInput")
    d_amask = nc.dram_tensor("atoms_mask", [G, N], F32, kind="ExternalInput")
    # ladjT_r[g, p, t, i] = additive mask for edge j->?  (j = t*128+p)
    d_ladjT = nc.dram_tensor("ladjT_r", [G, 128, NT, N], F32,
                             kind="ExternalInput")
    d_amino = nc.dram_tensor("amino_f", [G, L], F32, kind="ExternalInput")
    d_pmask = nc.dram_tensor("amino_mask", [G, L], F32, kind="ExternalInput")
    d_Eat = nc.dram_tensor("E_atom_pad", [128, CD], MD, kind="ExternalInput")
    d_Eam = nc.dram_tensor("E_amino", [NAM, PD], MD, kind="ExternalInput")
    # W_gat_r[p, h, q] = W_gat[h, p, q]
    d_Wg = nc.dram_tensor("W_gat_r", [CD, H, GD], MD, kind="ExternalInput")
    # Wa12[p, h, s]: s=0 -> (W_gat[h] @ a1_h)[p], s=1 -> (W_gat[h] @ a2_h)[p]
    d_Wa12 = nc.dram_tensor("Wa12", [CD, H, 2], MD, kind="ExternalInput")
    # a2_rows[0, h, q] = a_gat[h, GD+q]; a2go_row[0, q] = a_go[CD+q]
    d_a2r = nc.dram_tensor("a2_rows", [1, H, GD], MD, kind="ExternalInput")
    d_a2go = nc.dram_tensor("a2go_row", [1, CD], MD, kind="ExternalInput")
    # W_go_r[p, c, q] = W_go[c*128+p, q]
    d_Wgo = nc.dram_tensor("W_go_r", [128, 2, CD], MD, kind="ExternalInput")
    # Wgoa[p, c, s] = (W_go @ a{s}_go)[c*128+p]
    d_Wgoa = nc.dram_tensor("Wgoa", [128, 2, 2], MD, kind="ExternalInput")
    d_Wc = nc.dram_tensor("W_comp_wT", [CD, LAT], MD, kind="ExternalInput")
    d_bc = nc.dram_tensor("W_comp_b", [LAT, 1], F32, kind="ExternalInput")
    # MiT_r[p, l, i, q] = band matrix MiT[l, i, p, q]
    d_MiT = nc.dram_tensor("MiT_r", [PD, LC, KW, PD], MD,
                           kind="ExternalInput")
    d_cb = nc.dram_tensor("conv_b", [LC, 1], F32, kind="ExternalInput")
    d_Wa = nc.dram_tensor("W_att_wT", [LAT, LAT], MD, kind="ExternalInput")
    d_ba = nc.dram_tensor("W_att_b", [LAT, 1], F32, kind="ExternalInput")
    d_pw = nc.dram_tensor("pw_cols", [LAT, 2], F32, kind="ExternalInput")
    d_pb = nc.dram_tensor("pred_b", [1, 1], F32, kind="ExternalInput")
    d_out = nc.dram_tensor("out", [G, 1], F32, kind="ExternalOutput")

    with tile.TileContext(nc) as tc:
        with (
            tc.tile_pool(name="const", bufs=1) as cpool,
            tc.tile_pool(name="work", bufs=1) as wpool,
            tc.tile_pool(name="big", bufs=2) as bpool,
            tc.tile_pool(name="adj", bufs=2) as apool,
            tc.tile_pool(name="rows", bufs=1) as rpool,
            tc.tile_pool(name="ps_sq", bufs=2, space="PSUM") as pssq,
            tc.tile_pool(name="ps_row", bufs=1, space="PSUM") as psrow,
            tc.tile_pool(name="ps_cv", bufs=4, space="PSUM") as pscv,
            tc.tile_pool(name="ps_wh", bufs=1, space="PSUM") as pswh,
        ):
            # ---- constants / weights resident in SBUF ----
            ioi = cpool.tile([128, L], I32)
            nc.gpsimd.iota(ioi, pattern=[[0, L]], base=0, channel_multiplier=1)
            iof = cpool.tile([128, L], F32)
            nc.vector.tensor_copy(iof, ioi)
            ones_col = cpool.tile([128, 1], F32)
            nc.vector.memset(ones_col, 1.0)
            ones_col_m = cpool.tile([128, 1], MD)
            nc.vector.memset(ones_col_m, 1.0)

            Eat = cpool.tile([128, CD], MD)
            nc.sync.dma_start(out=Eat, in_=d_Eat[:, :])
            Eam = cpool.tile([NAM, PD], MD)
            nc.sync.dma_start(out=Eam, in_=d_Eam[:, :])
            Wg = cpool.tile([CD, H, GD], MD)
            nc.sync.dma_start(out=Wg, in_=d_Wg[:, :, :])
            Wa12 = cpool.tile([CD, H, 2], MD)
            nc.sync.dma_start(out=Wa12, in_=d_Wa12[:, :, :])
            a2r = cpool.tile([1, H, GD], MD)
            nc.sync.dma_start(out=a2r, in_=d_a2r[:, :, :])
            a2go = cpool.tile([1, CD], MD)
            nc.sync.dma_start(out=a2go, in_=d_a2go[:, :])
            Wgo = cpool.tile([128, 2, CD], MD)
            nc.sync.dma_start(out=Wgo, in_=d_Wgo[:, :, :])
            Wgoa = cpool.tile([128, 2, 2], MD)
            nc.sync.dma_start(out=Wgoa, in_=d_Wgoa[:, :, :])
            Wc = cpool.tile([CD, LAT], MD)
            nc.sync.dma_start(out=Wc, in_=d_Wc[:, :])
            bc = cpool.tile([LAT, 1], F32)
            nc.sync.dma_start(out=bc, in_=d_bc[:, :])
            MiT = cpool.tile([PD, LC, KW, PD], MD)
            nc.scalar.dma_start(out=MiT, in_=d_MiT[:, :, :, :])
            cb = cpool.tile([128, LC], F32)
            nc.sync.dma_start(
                out=cb,
                in_=bass.AP(tensor=d_cb, offset=0, ap=[[0, 128], [1, LC], [0, 1]]),
            )
            Wa = cpool.tile([LAT, LAT], MD)
            nc.scalar.dma_start(out=Wa, in_=d_Wa[:, :])
            ba = cpool.tile([LAT, 1], F32)
            nc.sync.dma_start(out=ba, in_=d_ba[:, :])
            pw = cpool.tile([LAT, 2], F32)
            nc.sync.dma_start(out=pw, in_=d_pw[:, :])
            pb = cpool.tile([1, 1], F32)
            nc.sync.dma_start(out=pb, in_=d_pb[:, :])

            def leaky(out, in_, alpha, bias=None):
                """out = leaky_relu(in_ + bias, alpha). in_ may be PSUM."""
                if use_prelu:
                    if bias is None:
                        nc.scalar.activation(out=out, in_=in_, func=AF.Prelu,
                                             alpha=alpha)
                    else:
                        nc.scalar.activation(out=out, in_=in_, func=AF.Prelu,
                                             bias=bias, alpha=alpha)
                    return
                src = in_
                if bias is not None:
                    t = wpool.tile(list(out.shape), F32, tag="t2k",
                                   bufs=6, name="lkb")
                    nc.scalar.activation(out=t, in_=in_, func=AF.Identity,
                                         bias=bias)
                    src = t
                nc.vector.scalar_tensor_tensor(
                    out=out, in0=src, scalar=alpha, in1=src,
                    op0=AT.mult, op1=AT.max)

            def elu_into(out_ap, hp_src, rb, m, tag_sfx):
                """out = elu(hp_src * rb); hp_src PSUM (m, N), rb SBUF (m, N)."""
                hpn = wpool.tile([m, N], F32, tag="t2k", bufs=6,
                                 name="hpn" + tag_sfx)
                nc.vector.scalar_tensor_tensor(
                    out=hpn, in0=hp_src, scalar=1.0, in1=rb,
                    op0=AT.mult, op1=AT.mult)
                xm = wpool.tile([m, N], F32, tag="t2k", bufs=6,
                                name="xm" + tag_sfx)
                nc.vector.tensor_scalar(out=xm, in0=hpn, scalar1=0.0,
                                        scalar2=None, op0=AT.min)
                em = wpool.tile([m, N], F32, tag="t2k", bufs=6,
                                name="em" + tag_sfx)
                nc.scalar.activation(out=em, in_=xm, func=AF.Exp)
                nc.vector.scalar_tensor_tensor(
                    out=out_ap, in0=em, scalar=-1.0, in1=hpn,
                    op0=AT.add, op1=AT.max)

            # per-graph state carried into the fused conv / tail phases
            st = [dict() for _ in range(G)]

            for g in range(G):
                # ---------- atom embeddings via one-hot matmul ----------
                arow = rpool.tile([1, N], F32, tag="r2k", bufs=4, name="arow")
                nc.sync.dma_start(out=arow, in_=d_atoms[g : g + 1, :])
                ab = wpool.tile([128, N], F32, tag="t2k", bufs=6, name="ab")
                nc.gpsimd.partition_broadcast(ab, arow)
                ohA = wpool.tile([128, N], MD, tag="t2k", bufs=6, name="ohA")
                nc.vector.tensor_tensor(out=ohA, in0=ab, in1=iof[:, :N],
                                        op=AT.is_equal)
                avT_ps = pssq.tile([128, N], F32, tag="mm_sq", name="avT_ps")
                nc.tensor.matmul(avT_ps, Eat, ohA, start=True, stop=True)
                avT = wpool.tile([128, N], MD, tag="avT", bufs=2, name="avT")
                nc.scalar.copy(avT, avT_ps)

                ladjT = apool.tile([128, NT, N], F32, tag="ladjT", name="ladjT")
                nc.gpsimd.dma_start(out=ladjT, in_=d_ladjT[g])

                m01 = wpool.tile([128, N], MD, tag="multi01", bufs=2, name="m01")
                m23 = wpool.tile([128, N], MD, tag="multi23", bufs=2, name="m23")
                multi = [m01, m23]

                def gat_attention(whsb, nk, src_ps, dcol, rowsum_sep,
                                  tag_pfx):
                    """z -> leaky -> exp -> hp (+rowsum) -> 1/rowsum bcast.

                    src_ps: PSUM (1, N) row; dcol: SBUF (128, NT) dst columns.
                    Returns (hp_ps, rb).
                    """
                    sd = rpool.tile([1, N], F32, tag="sd", bufs=2, name="sd")
                    nc.vector.tensor_copy(sd, src_ps)
                    srcb = wpool.tile([128, N], F32, tag="t2k", bufs=6,
                                      name="srcb" + tag_pfx)
                    nc.gpsimd.partition_broadcast(srcb, sd)

                    # z = src_bcast + dst + ladj  (one fused DVE op per chunk)
                    zm = bpool.tile([128, NT, N], F32, tag="zm", name="zm")
                    for t in range(NT):
                        nc.vector.scalar_tensor_tensor(
                            out=zm[:, t, :], in0=srcb,
                            scalar=dcol[:, t : t + 1],
                            in1=ladjT[:, t, :], op0=AT.add, op1=AT.add)
                    ee = bpool.tile([128, NT, N], F32, tag="ee", name="ee")
                    leaky(ee, zm, ALPHA)
                    U = bpool.tile([128, NT, N], MD, tag="U", name="U")
                    nc.scalar.activation(out=U, in_=ee, func=AF.Exp)
                    hp_m = nk + (0 if rowsum_sep else 1)
                    hp_ps = pssq.tile([128, N], F32, tag="mm_sq", name="hp_ps")
                    for t in range(NT):
                        nc.tensor.matmul(hp_ps[:hp_m, :],
                                         whsb[:, t, :], U[:, t, :],
                                         start=(t == 0), stop=(t == NT - 1))
                    if rowsum_sep:
                        rs_ps = psrow.tile([1, N], F32, tag="ps_row",
                                           name="rs_ps")
                        for t in range(NT):
                            nc.tensor.matmul(rs_ps, ones_col_m, U[:, t, :],
                                             start=(t == 0), stop=(t == NT - 1))
                        zrow = rs_ps
                    else:
                        zrow = hp_ps[nk : nk + 1, :]
                    zrw = rpool.tile([1, N], F32, tag="r2k", bufs=4, name="zrw")
                    nc.vector.tensor_copy(zrw, zrow)
                    rinv = rpool.tile([1, N], F32, tag="r2k", bufs=4, name="rinv")
                    scr = rpool.tile([1, N], F32, tag="r2k", bufs=4, name="rscr")
                    nc.vector.reciprocal_approx_accurate(out=rinv, in_=zrw,
                                                         scratch=scr)
                    rb = wpool.tile([nk, N], F32, tag="t2k", bufs=6,
                                    name="rb" + tag_pfx)
                    nc.gpsimd.partition_broadcast(rb, rinv)
                    return hp_ps, rb

                # ---------- GAT heads ----------
                # all heads' Wh chunks, batched so each avT chunk (lhsT) is
                # loaded once and reused for the 4 heads
                whsb_all = wpool.tile([128, NT, H, GD + 1], MD,
                                      tag="whsb_all", bufs=1, name="whsb_all")
                nc.vector.memset(whsb_all, 1.0)
                for half in range(2):
                    wh_all = pswh.tile([128, 2, H, GD], F32, tag="wh_all",
                                       name="wh_all")
                    for t2 in range(2):
                        t = half * 2 + t2
                        for h in range(H):
                            nc.tensor.matmul(
                                wh_all[:, t2, h, :],
                                avT[:, t * 128 : (t + 1) * 128],
                                Wg[:, h, :], start=True, stop=True)
                    nc.scalar.copy(
                        whsb_all[:, half * 2 : half * 2 + 2, :, :GD], wh_all)

                for h in range(H):
                    whsb = whsb_all[:, :, h, :]
                    src_ps = psrow.tile([1, N], F32, tag="ps_row",
                                        name="src_ps")
                    nc.tensor.matmul(src_ps, Wa12[:, h, 0:1], avT,
                                     start=True, stop=True)
                    a2b = wpool.tile([128, GD], MD, tag="a2b", bufs=2,
                                     name="a2b")
                    nc.gpsimd.partition_broadcast(a2b, a2r[0:1, h, :])
                    dcol = rpool.tile([128, NT], F32, tag="dcol", bufs=2,
                                      name="dcol")
                    dsc = rpool.tile([128, GD], F32, tag="dsc", bufs=2,
                                     name="dsc")
                    for t in range(NT):
                        nc.vector.scalar_tensor_tensor(
                            out=dsc, in0=whsb[:, t, :GD], scalar=1.0,
                            in1=a2b, op0=AT.mult, op1=AT.mult,
                            accum_out=dcol[:, t : t + 1])

                    hp_ps, rb = gat_attention(whsb, GD, src_ps, dcol,
                                              False, "h")
                    dsti = multi[h // 2]
                    off = (h % 2) * GD
                    elu_into(dsti[off : off + GD, :], hp_ps[:GD, :], rb,
                             GD, "h")

                # ---------- GAT output layer ----------
                wh2_ps = pssq.tile([128, NT, CD], F32, tag="mm_sq",
                                   name="wh2_ps")
                for t in range(NT):
                    for c in range(2):
                        nc.tensor.matmul(
                            wh2_ps[:, t, :],
                            multi[c][:, t * 128 : (t + 1) * 128],
                            Wgo[:, c, :], start=(c == 0), stop=(c == 1))
                wh2sb = wpool.tile([128, NT, CD], MD, tag="t2k", bufs=6,
                                   name="wh2sb")
                nc.scalar.copy(wh2sb, wh2_ps)

                src2_ps = psrow.tile([1, N], F32, tag="ps_row",
                                     name="src2_ps")
                for c in range(2):
                    nc.tensor.matmul(src2_ps, Wgoa[:, c, 0:1], multi[c],
                                     start=(c == 0), stop=(c == 1))
                a2gob = wpool.tile([128, CD], MD, tag="a2b", bufs=2,
                                   name="a2gob")
                nc.gpsimd.partition_broadcast(a2gob, a2go)
                dcol2 = rpool.tile([128, NT], F32, tag="dcol", bufs=2,
                                   name="dcol2")
                dsc2 = rpool.tile([128, CD], F32, tag="dsc", bufs=2,
                                  name="dsc2")
                for t in range(NT):
                    nc.vector.scalar_tensor_tensor(
                        out=dsc2, in0=wh2sb[:, t, :], scalar=1.0,
                        in1=a2gob, op0=AT.mult, op1=AT.mult,
                        accum_out=dcol2[:, t : t + 1])

                hp2_ps, rb2 = gat_attention(wh2sb, CD, src2_ps, dcol2,
                                            True, "o")
                xT = wpool.tile([CD, N], MD, tag="xT", bufs=2, name="xT")
                elu_into(xT, hp2_ps, rb2, CD, "o")

                # ---------- atoms_vec -> a_v -> comp pooling ----------
                av_ps = pssq.tile([LAT, N], F32, tag="mm_sq", name="av_ps")
                nc.tensor.matmul(av_ps, Wc, xT, start=True, stop=True)
                avec = wpool.tile([LAT, N], MD, tag="t2k", bufs=6, name="avec")
                leaky(avec, av_ps, ALPHA, bias=bc)
                av2_ps = pssq.tile([LAT, N], F32, tag="mm_sq", name="av2_ps")
                nc.tensor.matmul(av2_ps, Wa, avec, start=True, stop=True)
                a_v = wpool.tile([LAT, N], F32, tag="t2k", bufs=6, name="a_v")
                leaky(a_v, av2_ps, ALPHA, bias=ba)

                amrow = rpool.tile([1, N], F32, tag="r2k", bufs=4, name="amrow")
                nc.sync.dma_start(out=amrow, in_=d_amask[g : g + 1, :])
                amb = wpool.tile([128, N], F32, tag="t2k", bufs=6, name="amb")
                nc.gpsimd.partition_broadcast(amb, amrow)
                cscr = wpool.tile([LAT, N], F32, tag="t2k", bufs=6, name="cscr")
                comp_acc = rpool.tile([LAT, 1], F32, tag="c1", bufs=4,
                                      name="comp_acc")
                nc.vector.scalar_tensor_tensor(
                    out=cscr, in0=a_v, scalar=1.0, in1=amb,
                    op0=AT.mult, op1=AT.mult, accum_out=comp_acc)
                amscr = rpool.tile([1, N], F32, tag="r2k", bufs=4, name="amscr")
                amsum = rpool.tile([1, 1], F32, tag="c2", bufs=8, name="amsum")
                nc.vector.tensor_scalar(out=amscr, in0=amrow, scalar1=1.0,
                                        scalar2=0.0, op0=AT.mult, op1=AT.add,
                                        accum_out=amsum)
                amsb = rpool.tile([128, 1], F32, tag="c2", bufs=8, name="amsb")
                nc.gpsimd.partition_broadcast(amsb, amsum)
                amr = rpool.tile([128, 1], F32, tag="c2", bufs=8, name="amr")
                nc.vector.reciprocal(amr, amsb)
                cp = rpool.tile([128, 2], F32, tag="cp", bufs=6, name="cp")
                nc.vector.tensor_scalar(out=cp[:, 0:1], in0=comp_acc,
                                        scalar1=amr, scalar2=None,
                                        op0=AT.mult)
                st[g]["cp"] = cp

                # ---------- protein embedding (conv input) ----------
                prow = rpool.tile([1, L], F32, tag="r4k", bufs=2, name="prow")
                nc.sync.dma_start(out=prow, in_=d_amino[g : g + 1, :])
                pbm = wpool.tile([128, L], F32, tag="t4k", bufs=3, name="pbm")
                nc.gpsimd.partition_broadcast(pbm, prow)
                ohP = wpool.tile([NAM, L], MD, tag="t4k", bufs=3, name="ohP")
                nc.vector.tensor_tensor(out=ohP, in0=pbm[:NAM, :],
                                        in1=iof[:NAM, :], op=AT.is_equal)
                pv = bpool.tile([PD, L + 2 * PADL], MD, tag=f"pv{g}_0",
                                bufs=1, name="pv")
                nc.vector.memset(pv[:, :PADL], 0.0)
                nc.vector.memset(pv[:, PADL + L :], 0.0)
                for nn in range(2):
                    pvT_ps = pscv.tile([PD, 512], F32, tag="ps_cv",
                                       name="pvT_ps")
                    nc.tensor.matmul(pvT_ps, Eam,
                                     ohP[:, nn * 512 : (nn + 1) * 512],
                                     start=True, stop=True)
                    nc.scalar.copy(
                        pv[:, PADL + nn * 512 : PADL + (nn + 1) * 512], pvT_ps)
                st[g]["pv"] = pv

            # ---------- conv layers, both graphs interleaved ----------
            # (shared MiT weights stay loaded across 4 consecutive matmuls)
            for lyr in range(LC):
                pvo_l = []
                for g in range(G):
                    pvo = bpool.tile([PD, L + 2 * PADL], MD,
                                     tag=f"pv{g}_{1 - lyr % 2}", bufs=1,
                                     name="pvo")
                    nc.vector.memset(pvo[:, :PADL], 0.0)
                    nc.vector.memset(pvo[:, PADL + L :], 0.0)
                    pvo_l.append(pvo)
                cv_ps = {}
                for g in range(G):
                    for nn in range(2):
                        cv_ps[(g, nn)] = pscv.tile([PD, 512], F32,
                                                   tag="ps_cv",
                                                   name=f"cv{g}{nn}")
                for i in range(KW):
                    for g in range(G):
                        pv = st[g]["pv"]
                        for nn in range(2):
                            nc.tensor.matmul(
                                cv_ps[(g, nn)], MiT[:, lyr, i, :],
                                pv[:, nn * 512 + i : nn * 512 + i + 512],
                                start=(i == 0), stop=(i == KW - 1))
                for g in range(G):
                    for nn in range(2):
                        nc.scalar.activation(
                            out=pvo_l[g][:, PADL + nn * 512 :
                                         PADL + (nn + 1) * 512],
                            in_=cv_ps[(g, nn)], func=AF.Relu,
                            bias=cb[:, lyr : lyr + 1])
                for g in range(G):
                    st[g]["pv"] = pvo_l[g]

            # ---------- p_v + prot pooling + head, per graph ----------
            for g in range(G):
                amv = st[g]["pv"][:, PADL : PADL + L]
                cp = st[g]["cp"]
                p_v = wpool.tile([LAT, L], F32, tag="t4k", bufs=3, name="p_v")
                for nn in range(2):
                    pv_ps = pscv.tile([LAT, 512], F32, tag="ps_cv",
                                      name="pv_ps")
                    nc.tensor.matmul(pv_ps, Wa,
                                     amv[:, nn * 512 : (nn + 1) * 512],
                                     start=True, stop=True)
                    leaky(p_v[:, nn * 512 : (nn + 1) * 512], pv_ps, ALPHA,
                          bias=ba)
                pmrow = rpool.tile([1, L], F32, tag="r4k", bufs=2, name="pmrow")
                nc.sync.dma_start(out=pmrow, in_=d_pmask[g : g + 1, :])
                pmb = wpool.tile([128, L], F32, tag="t4k", bufs=3, name="pmb")
                nc.gpsimd.partition_broadcast(pmb, pmrow)
                pscr = wpool.tile([LAT, L], F32, tag="t4k", bufs=3, name="pscr")
                prot_acc = rpool.tile([LAT, 1], F32, tag="c1", bufs=4,
                                      name="prot_acc")
                nc.vector.scalar_tensor_tensor(
                    out=pscr, in0=p_v, scalar=1.0, in1=pmb,
                    op0=AT.mult, op1=AT.mult, accum_out=prot_acc)
                pmscr = rpool.tile([1, L], F32, tag="r4k", bufs=2, name="pmscr")
                pmsum = rpool.tile([1, 1], F32, tag="c2", bufs=8, name="pmsum")
                nc.vector.tensor_scalar(out=pmscr, in0=pmrow, scalar1=1.0,
                                        scalar2=0.0, op0=AT.mult, op1=AT.add,
                                        accum_out=pmsum)
                pmsb = rpool.tile([128, 1], F32, tag="c2", bufs=8, name="pmsb")
                nc.gpsimd.partition_broadcast(pmsb, pmsum)
                pmr = rpool.tile([128, 1], F32, tag="c2", bufs=8, name="pmr")
                nc.vector.reciprocal(pmr, pmsb)
                nc.vector.tensor_scalar(out=cp[:, 1:2], in0=prot_acc,
                                        scalar1=pmr, scalar2=None,
                                        op0=AT.mult)

                lr2 = rpool.tile([128, 2], F32, tag="cp", bufs=6, name="lr2")
                leaky(lr2, cp, ALPHA * ALPHA)
                dscr = rpool.tile([128, 2], F32, tag="cp", bufs=6, name="dscr")
                dacc = rpool.tile([128, 1], F32, tag="c1", bufs=4, name="dacc")
                nc.vector.scalar_tensor_tensor(
                    out=dscr, in0=lr2, scalar=1.0, in1=pw,
                    op0=AT.mult, op1=AT.mult, accum_out=dacc)
                fin_ps = psrow.tile([1, 1], F32, tag="ps_row", name="fin_ps")
                nc.tensor.matmul(fin_ps, dacc, ones_col, start=True, stop=True)
                res = rpool.tile([1, 1], F32, tag="c2", bufs=8, name="res")
                nc.scalar.activation(out=res, in_=fin_ps, func=AF.Identity,
                                     bias=pb)
                nc.sync.dma_start(out=d_out[g : g + 1, :], in_=res)

    return nc


def preprocess(inputs, mm_bf16=True):
    """Host-side prep: shard over cores, transpose/reshape weights."""
    import ml_dtypes
    md = ml_dtypes.bfloat16 if mm_bf16 else np.float32
    atoms = np.asarray(inputs["atoms"]).astype(np.float32)
    atoms_mask = np.asarray(inputs["atoms_mask"]).astype(np.float32)
    adjacency = np.asarray(inputs["adjacency"])
    amino = np.asarray(inputs["amino"]).astype(np.float32)
    amino_mask = np.asarray(inputs["amino_mask"]).astype(np.float32)
    E_atom = np.asarray(inputs["E_atom"]).astype(np.float32)
    E_amino = np.asarray(inputs["E_amino"]).astype(np.float32)
    W_gat = np.asarray(inputs["W_gat"]).astype(np.float32)
    a_gat = np.asarray(inputs["a_gat"]).astype(np.float32)
    W_go = np.asarray(inputs["W_go"]).astype(np.float32)
    a_go = np.asarray(inputs["a_go"]).astype(np.float32)
    W_comp_w = np.asarray(inputs["W_comp_w"]).astype(np.float32)
    W_comp_b = np.asarray(inputs["W_comp_b"]).astype(np.float32)
    conv_w = np.asarray(inputs["conv_w"]).astype(np.float32)
    conv_b = np.asarray(inputs["conv_b"]).astype(np.float32)
    W_att_w = np.asarray(inputs["W_att_w"]).astype(np.float32)
    W_att_b = np.asarray(inputs["W_att_b"]).astype(np.float32)
    pred_w = np.asarray(inputs["pred_w"]).astype(np.float32)
    pred_b = np.asarray(inputs["pred_b"]).astype(np.float32)

    # additive mask, transposed, pre-tiled: [g, p, t, i] = mask(j=t*128+p, i)
    ladjT = np.where(adjacency.transpose(0, 2, 1) > 0, np.float32(0.0),
                     np.float32(MASKNEG)).astype(np.float32)
    ladjT_r = np.ascontiguousarray(
        ladjT.reshape(B, NT, 128, N).transpose(0, 2, 1, 3))

    E_atom_pad = np.zeros((128, CD), np.float32)
    E_atom_pad[:NA] = E_atom

    # conv band matrices: MiT[l, i, din, dout] = conv_w[l,0,0,i, din-dout+5]
    MiT = np.zeros((LC, KW, PD, PD), np.float32)
    din = np.arange(PD)[:, None]
    dout = np.arange(PD)[None, :]
    v = din - dout + (KW // 2)
    valid = (v >= 0) & (v < KW)
    vc = np.clip(v, 0, KW - 1)
    for lyr in range(LC):
        for i in range(KW):
            MiT[lyr, i] = np.where(valid, conv_w[lyr, 0, 0, i, vc], 0.0)
    MiT_r = np.ascontiguousarray(MiT.transpose(2, 0, 1, 3))

    W_gat_r = np.ascontiguousarray(W_gat.transpose(1, 0, 2))
    # Wa12[p, h, 0] = (W_gat[h] @ a1_h)[p]
    Wa1 = np.einsum("hpq,hq->ph", W_gat, a_gat[:, :GD])
    Wa2 = np.einsum("hpq,hq->ph", W_gat, a_gat[:, GD:])
    Wa12 = np.ascontiguousarray(np.stack([Wa1, Wa2], axis=2))
    W_go_r = np.ascontiguousarray(
        W_go.reshape(2, 128, CD).transpose(1, 0, 2))
    Wgoa = np.stack([W_go @ a_go[:CD], W_go @ a_go[CD:]], axis=1)  # (256, 2)
    Wgoa_r = np.ascontiguousarray(
        Wgoa.reshape(2, 128, 2).transpose(1, 0, 2))

    shared = {
        "E_atom_pad": E_atom_pad.astype(md),
        "E_amino": np.ascontiguousarray(E_amino).astype(md),
        "W_gat_r": W_gat_r.astype(md),
        "Wa12": Wa12.astype(md),
        "a2_rows": np.ascontiguousarray(a_gat[:, GD:][None, :, :]).astype(md),
        "a2go_row": np.ascontiguousarray(a_go[CD:][None, :]).astype(md),
        "W_go_r": W_go_r.astype(md),
        "Wgoa": Wgoa_r.astype(md),
        "W_comp_wT": np.ascontiguousarray(W_comp_w.T).astype(md),
        "W_comp_b": np.ascontiguousarray(W_comp_b[:, None]),
        "MiT_r": MiT_r.astype(md),
        "conv_b": np.ascontiguousarray(conv_b.reshape(LC, 1)),
        "W_att_wT": np.ascontiguousarray(W_att_w.T).astype(md),
        "W_att_b": np.ascontiguousarray(W_att_b[:, None]),
        "pw_cols": np.ascontiguousarray(
            np.stack([pred_w[0, :LAT], pred_w[0, LAT:]], axis=1)),
        "pred_b": np.ascontiguousarray(pred_b.reshape(1, 1)),
    }
    in_maps = []
    for c in range(NCORES):
        sl = slice(c * G, (c + 1) * G)
        m = dict(shared)
        m["atoms_f"] = np.ascontiguousarray(atoms[sl])
        m["atoms_mask"] = np.ascontiguousarray(atoms_mask[sl])
        m["ladjT_r"] = np.ascontiguousarray(ladjT_r[sl])
        m["amino_f"] = np.ascontiguousarray(amino[sl])
        m["amino_mask"] = np.ascontiguousarray(amino_mask[sl])
        in_maps.append(m)
    return in_maps


_CACHED_NC = None


def kernel(**inputs) -> np.ndarray:
    global _CACHED_NC
    from concourse.bass_utils import run_bass_kernel_spmd

    if _CACHED_NC is None:
        nc = build_core_program()
        nc.finalize()
        _CACHED_NC = nc
    nc = _CACHED_NC
    in_maps = preprocess(inputs)
    res = run_bass_kernel_spmd(nc, in_maps, core_ids=list(range(NCORES)))
    out = np.concatenate([res.results[c]["out"] for c in range(NCORES)], axis=0)
    return out.astype(np.float32)
